# revision 1
# baseline (speedup 1.0000x reference)
"""LlamaAttention (B=2, S=2048, D=2048, H=16) on 8 Trainium2 NeuronCores.

Sharding: batch x head-group. Core c handles batch b = c // 4 and head group
g = c % 4 (4 heads of 128 dims each -> a 512-wide slice of q/k/v space).
Each core computes q/k/v projections for its slice, attention for its 4
heads, and a partial out-projection (contracting only its 512 dv dims).
Host sums the 4 partials per batch and adds the output bias.

Device layout notes (all fp32):
  - x is staged transposed: xT [d, s] so the d contraction sits on SBUF
    partitions for the projection matmuls.
  - q, k are produced transposed (qT/kT [e, s]); v in natural layout [s, e].
  - scores are computed transposed: sT[sk, sq] = kT.T-slice @ qT, so the
    softmax key-reduction lives on the partition axis. exp() is applied by
    the scalar engine straight out of PSUM, with the additive attention
    mask folded in as the activation's per-partition bias (mask is per-key,
    keys are partitions in this layout -> exact general mask for free).
  - softmax denominator r[sq] = ones-vector matmul over exp tiles (partition
    reduction on the PE), reciprocal on DVE, broadcast via GpSimd,
    normalization fused into the PV-psum eviction on DVE.
  - PV is computed transposed as well: oT[dv, sq] = v-slice.T @ expT, which
    feeds the out-projection directly (dv contraction on partitions).
  - no max-subtraction in softmax: scores are O(3) for this problem scale
    (|q.k| ~ N(0,1)-ish), exp is evaluated in fp32 with <=2 ULP error.
"""

import os
import numpy as np

import concourse.bass as bass
import concourse.tile as tile
from concourse import bacc, mybir
from concourse import bass_utils

B, S, D = 2, 2048, 2048
NH, HD = 16, 128
N_CORES = 8
HPC = 4                      # heads per core
E = HPC * HD                 # 512: per-core q/k/v width
SCALE = float(HD) ** -0.5
F32 = mybir.dt.float32

P = 128                      # partition tile
ST = S // P                  # 16 s partition-tiles
DTI = D // P                 # 16 d partition-tiles
ETI = E // P                 # 4 e partition-tiles (= heads per core)
SB = 512                     # matmul moving-dim block
NBLK = S // SB               # 4 s blocks
QKCH = 256                   # s-chunk width for the q/k projection pass
MASK_MIN = float(np.finfo(np.float32).min)

# matmul input dtype: float32 (exact-ish) or float32r (4x faster PE)
_MM_DT_ENV = os.environ.get("BASS_MM_DT", "fp32r")
MM_DT = mybir.dt.float32r if _MM_DT_ENV == "fp32r" else mybir.dt.float32


def _build(has_bias: bool):
    nc = bacc.Bacc("TRN2", target_bir_lowering=False, debug=False,
                   num_devices=N_CORES)

    xT = nc.dram_tensor("xT", [D, S], MM_DT, kind="ExternalInput").ap()
    wqT = nc.dram_tensor("wqT", [D, E], MM_DT, kind="ExternalInput").ap()
    wkT = nc.dram_tensor("wkT", [D, E], MM_DT, kind="ExternalInput").ap()
    wvT = nc.dram_tensor("wvT", [D, E], MM_DT, kind="ExternalInput").ap()
    woT = nc.dram_tensor("woT", [E, D], MM_DT, kind="ExternalInput").ap()
    maskT = nc.dram_tensor("maskT", [S], F32, kind="ExternalInput").ap()
    ones1 = nc.dram_tensor("ones1", [SB], MM_DT, kind="ExternalInput").ap()
    if has_bias:
        bqd = nc.dram_tensor("bq", [E], MM_DT, kind="ExternalInput").ap()
        bkd = nc.dram_tensor("bk", [E], MM_DT, kind="ExternalInput").ap()
        bvd = nc.dram_tensor("bv", [E], MM_DT, kind="ExternalInput").ap()
    yT = nc.dram_tensor("yT", [D, S], F32, kind="ExternalOutput").ap()

    with tile.TileContext(nc) as tc:
        with tc.tile_pool(name="persist", bufs=1) as persist:
            qT = [persist.tile([P, S], MM_DT, name=f"qT{i}", tag=f"qT{i}")
                  for i in range(ETI)]
            kT = [persist.tile([P, S], MM_DT, name=f"kT{i}", tag=f"kT{i}")
                  for i in range(ETI)]
            vv = [persist.tile([P, E], MM_DT, name=f"v{i}", tag=f"v{i}")
                  for i in range(ST)]
            mask_sb = persist.tile([P, ST], F32, name="mask_sb", tag="mask")
            nc.sync.dma_start(mask_sb[:, :],
                              maskT.rearrange("(t p) -> p t", p=P))
            ones_col = persist.tile([P, 1], MM_DT, name="ones_col", tag="onesc")
            nc.sync.dma_start(ones_col[:, :],
                              ones1[0:P].rearrange("(p a) -> p a", a=1))
            if has_bias:
                ones_row = persist.tile([1, SB], MM_DT, name="ones_row",
                                        tag="onesr")
                nc.sync.dma_start(ones_row[:, :],
                                  ones1.rearrange("(a e) -> a e", a=1))
                ones_rp = persist.tile([1, P], MM_DT, name="ones_rp",
                                       tag="onesrp")
                nc.sync.dma_start(ones_rp[:, :],
                                  ones1[0:P].rearrange("(a e) -> a e", a=1))
                bq_sb = persist.tile([1, E], MM_DT, name="bq_sb", tag="bq")
                bk_sb = persist.tile([1, E], MM_DT, name="bk_sb", tag="bk")
                bv_sb = persist.tile([1, E], MM_DT, name="bv_sb", tag="bv")
                nc.sync.dma_start(bq_sb[:, :], bqd.rearrange("(a e) -> a e", a=1))
                nc.sync.dma_start(bk_sb[:, :], bkd.rearrange("(a e) -> a e", a=1))
                nc.sync.dma_start(bv_sb[:, :], bvd.rearrange("(a e) -> a e", a=1))

            # ---------------- Phase A1: q and k projections ----------------
            # qT[e, s] = (wqT.T-slice @ xT) ( + bq ) * SCALE; kT likewise.
            # One pass per projection so weights stay resident and the
            # moving dim is a full 512 (amortizes the per-matmul self-load).
            for which in ("q", "k"):
                wdram = wqT if which == "q" else wkT
                outT = qT if which == "q" else kT
                with nc.named_scope(f"proj_{which}"), \
                     tc.tile_pool(name=f"w{which}", bufs=1) as wpool, \
                     tc.tile_pool(name=f"x{which}", bufs=1) as xpool, \
                     tc.tile_pool(name=f"ps_{which}", bufs=4,
                                  space="PSUM") as psa:
                    w_sb = [[None] * ETI for _ in range(DTI)]
                    for dt in range(DTI):
                        for et in range(ETI):
                            w_t = wpool.tile([P, P], MM_DT,
                                             name=f"w{which}_{dt}_{et}",
                                             tag=f"w{which}_{dt}_{et}")
                            nc.sync.dma_start(
                                w_t[:, :],
                                wdram[dt * P:(dt + 1) * P, et * P:(et + 1) * P])
                            w_sb[dt][et] = w_t
                    for ch in range(NBLK):
                        c0 = ch * SB
                        xc = []
                        for dt in range(DTI):
                            xt = xpool.tile([P, SB], MM_DT, name=f"x{which}_{dt}",
                                            tag=f"x{which}_{dt}")
                            nc.sync.dma_start(
                                xt[:, :], xT[dt * P:(dt + 1) * P, c0:c0 + SB])
                            xc.append(xt)
                        for et in range(ETI):
                            ps = psa.tile([P, SB], F32, name=f"ps_{which}t")
                            for dt in range(DTI):
                                nc.tensor.matmul(
                                    ps[:, :], w_sb[dt][et][:, :],
                                    xc[dt][:, :],
                                    start=(dt == 0),
                                    stop=(dt == DTI - 1 and not has_bias))
                            if has_bias:
                                bsb = bq_sb if which == "q" else bk_sb
                                nc.tensor.matmul(
                                    ps[:, :],
                                    bsb[0:1, et * P:(et + 1) * P],
                                    ones_row[0:1, 0:SB],
                                    start=False, stop=True)
                            if which == "q":
                                nc.scalar.mul(
                                    outT[et][:, c0:c0 + SB], ps[:, :], SCALE)
                            else:
                                nc.scalar.copy(
                                    outT[et][:, c0:c0 + SB], ps[:, :])

            # ---------------- Phase A2: v projection ----------------
            # v[s, e] = xT-slice.T @ wvT ( + bv ), natural layout.
            with nc.named_scope("proj_v"), \
                 tc.tile_pool(name="wv", bufs=1) as wvpool, \
                 tc.tile_pool(name="xv", bufs=1) as xvpool, \
                 tc.tile_pool(name="ps_v", bufs=4, space="PSUM") as psv:
                wv_sb = []
                for dt in range(DTI):
                    wv_t = wvpool.tile([P, E], MM_DT, name=f"wv_{dt}",
                                       tag=f"wv_{dt}")
                    nc.sync.dma_start(wv_t[:, :], wvT[dt * P:(dt + 1) * P, :])
                    wv_sb.append(wv_t)
                for ch in range(NBLK):
                    c0 = ch * SB
                    xc = []
                    for dt in range(DTI):
                        xt = xvpool.tile([P, SB], MM_DT, name=f"xv_{dt}",
                                         tag=f"xv_{dt}")
                        nc.sync.dma_start(
                            xt[:, :], xT[dt * P:(dt + 1) * P, c0:c0 + SB])
                        xc.append(xt)
                    for sl in range(SB // P):
                        st = ch * (SB // P) + sl
                        ps = psv.tile([P, E], F32, name="ps_vt")
                        for dt in range(DTI):
                            nc.tensor.matmul(
                                ps[:, :],
                                xc[dt][:, sl * P:(sl + 1) * P],
                                wv_sb[dt][:, :],
                                start=(dt == 0),
                                stop=(dt == DTI - 1 and not has_bias))
                        if has_bias:
                            nc.tensor.matmul(
                                ps[:, :], ones_rp[0:1, :],
                                bv_sb[0:1, :],
                                start=False, stop=True)
                        nc.vector.tensor_copy(vv[st][:, :], ps[:, :])

            # ---------------- Phase B + C: attention + out-projection ------
            with nc.named_scope("attn"), \
                 tc.tile_pool(name="otn", bufs=1) as opool, \
                 tc.tile_pool(name="expp", bufs=18) as expp, \
                 tc.tile_pool(name="smx", bufs=2) as smx, \
                 tc.tile_pool(name="wo", bufs=2) as wop, \
                 tc.tile_pool(name="stage", bufs=3) as stagep, \
                 tc.tile_pool(name="ps_sc", bufs=2, space="PSUM") as ps_sc, \
                 tc.tile_pool(name="ps_r", bufs=2, space="PSUM") as ps_r, \
                 tc.tile_pool(name="ps_o", bufs=2, space="PSUM") as ps_o, \
                 tc.tile_pool(name="ps_y", bufs=2, space="PSUM") as ps_y:
                oTn = [opool.tile([P, S], MM_DT, name=f"oTn{h}", tag=f"oTn{h}")
                       for h in range(HPC)]
                for blk in range(NBLK):
                    q0 = blk * SB
                    for h in range(HPC):
                        # scores^T (one K=128 matmul per key tile) -> exp
                        ex = []
                        for sk in range(ST):
                            ps = ps_sc.tile([P, SB], F32, name="ps_sct")
                            nc.tensor.matmul(
                                ps[:, :],
                                kT[h][:, sk * P:(sk + 1) * P],
                                qT[h][:, q0:q0 + SB],
                                start=True, stop=True)
                            ext = expp.tile([P, SB], MM_DT, name="ext")
                            nc.scalar.activation(
                                ext[:, :], ps[:, :],
                                mybir.ActivationFunctionType.Exp,
                                bias=mask_sb[:, sk:sk + 1], scale=1.0)
                            ex.append(ext)
                        # softmax denominator: r[sq] = sum_sk exp.
                        # Partial sums on DVE (frees the PE), one final
                        # ones-matmul for the cross-partition reduction.
                        racc_f = smx.tile([P, SB], F32, name="racc_f")
                        nc.vector.tensor_add(racc_f[:, :],
                                             ex[0].bitcast(F32)[:, :],
                                             ex[1].bitcast(F32)[:, :])
                        for sk in range(2, ST):
                            nc.vector.tensor_add(racc_f[:, :], racc_f[:, :],
                                                 ex[sk].bitcast(F32)[:, :])
                        racc_r = smx.tile([P, SB], MM_DT, name="racc_r")
                        nc.vector.tensor_copy(racc_r[:, :], racc_f[:, :])
                        rps = ps_r.tile([1, SB], F32, name="rps")
                        nc.tensor.matmul(rps[:, :], ones_col[:, :],
                                         racc_r[:, :], start=True, stop=True)
                        rcp = smx.tile([1, SB], F32, name="rcp")
                        nc.vector.reciprocal(rcp[:, :], rps[:, :])
                        rbc = smx.tile([P, SB], F32, name="rbc")
                        nc.gpsimd.partition_broadcast(rbc[:, :], rcp[0:1, :])
                        # oT[dv, sq] = v-slice.T @ expT, normalized on evict
                        ops = ps_o.tile([P, SB], F32, name="ops")
                        for sk in range(ST):
                            nc.tensor.matmul(
                                ops[:, :],
                                vv[sk][:, h * P:(h + 1) * P],
                                ex[sk][:, :],
                                start=(sk == 0), stop=(sk == ST - 1))
                        nc.vector.tensor_mul(
                            oTn[h][:, q0:q0 + SB], ops[:, :], rbc[:, :])
                    # out-projection for this s block
                    for eo in range(DTI):
                        wts = []
                        for dv in range(HPC):
                            wt = wop.tile([P, P], MM_DT, name="wo_t",
                                          tag=f"wo_{dv}")
                            nc.sync.dma_start(
                                wt[:, :],
                                woT[dv * P:(dv + 1) * P, eo * P:(eo + 1) * P])
                            wts.append(wt)
                        yps = ps_y.tile([P, SB], F32, name="yps")
                        for dv in range(HPC):
                            nc.tensor.matmul(
                                yps[:, :], wts[dv][:, :],
                                oTn[dv][:, q0:q0 + SB],
                                start=(dv == 0), stop=(dv == HPC - 1))
                        stg = stagep.tile([P, SB], F32, name="stg")
                        nc.vector.tensor_copy(stg[:, :], yps[:, :])
                        nc.sync.dma_start(
                            yT[eo * P:(eo + 1) * P, q0:q0 + SB], stg[:, :])

    nc.compile()
    return nc


_NC_CACHE = {}


def _get_nc(has_bias: bool):
    key = (has_bias, MM_DT)
    if key not in _NC_CACHE:
        _NC_CACHE[key] = _build(has_bias)
    return _NC_CACHE[key]


def kernel(hidden_states, attention_mask, Wq, bq, Wk, bk, Wv, bv, Wo, bo):
    hidden_states = np.asarray(hidden_states, dtype=np.float32)
    attention_mask = np.asarray(attention_mask, dtype=np.float32)
    Wq = np.asarray(Wq, dtype=np.float32)
    Wk = np.asarray(Wk, dtype=np.float32)
    Wv = np.asarray(Wv, dtype=np.float32)
    Wo = np.asarray(Wo, dtype=np.float32)
    bq = np.asarray(bq, dtype=np.float32)
    bk = np.asarray(bk, dtype=np.float32)
    bv = np.asarray(bv, dtype=np.float32)
    bo = np.asarray(bo, dtype=np.float32)

    has_bias = bool(np.any(bq) or np.any(bk) or np.any(bv))
    nc = _get_nc(has_bias)

    # Host-side sharding prep (cheap numpy work, not on the HW critical path)
    xT = [np.ascontiguousarray(hidden_states[b].T) for b in range(B)]
    addmask = [np.ascontiguousarray((1.0 - attention_mask[b]) * MASK_MIN)
               for b in range(B)]
    in_maps = []
    for c in range(N_CORES):
        b, g = c // 4, c % 4
        sl = slice(g * E, (g + 1) * E)
        im = {
            "xT": xT[b],
            "wqT": np.ascontiguousarray(Wq[sl, :].T),
            "wkT": np.ascontiguousarray(Wk[sl, :].T),
            "wvT": np.ascontiguousarray(Wv[sl, :].T),
            "woT": np.ascontiguousarray(Wo[:, sl].T),
            "maskT": addmask[b],
            "ones1": np.ones(SB, dtype=np.float32),
        }
        if has_bias:
            im["bq"] = np.ascontiguousarray(bq[sl])
            im["bk"] = np.ascontiguousarray(bk[sl])
            im["bv"] = np.ascontiguousarray(bv[sl])
        in_maps.append(im)

    res = bass_utils.run_bass_kernel_spmd(
        nc, in_maps, core_ids=list(range(N_CORES)),
        trace=bool(int(os.environ.get("BASS_KERNEL_TRACE", "0"))))
    kernel.last_results = res

    out = np.empty((B, S, D), dtype=np.float32)
    for b in range(B):
        acc = res.results[b * 4]["yT"].copy()
        for g in range(1, 4):
            acc += res.results[b * 4 + g]["yT"]
        out[b] = acc.T + bo
    return out



# revision 10
# speedup vs baseline: 1.5238x; 1.5238x over previous
"""LlamaAttention (B=2, S=2048, D=2048, H=16) on 8 Trainium2 NeuronCores.

Sharding: batch x head-group. Core c handles batch b = c // 4 and head group
g = c % 4 (4 heads of 128 dims each -> a 512-wide slice of q/k/v space).
Each core computes q/k/v projections for its slice, attention for its 4
heads, and a partial out-projection (contracting only its 512 dv dims).
Host sums the 4 partials per batch and adds the output bias.

v2 design (vs the fp32r v1 at ~812us):
  - all matmul inputs bf16: TRN2 PE streams bf16 at 1 cycle/row vs 2 for
    fp32r (fp32_mode=HIGH) -> halves the Tensor-engine time, which the v1
    trace showed 100% busy. PSUM accumulation stays fp32. Expected extra
    error ~0.3-0.5% rel, well under the 2e-2 gate.
  - x (hidden transposed) is loaded into SBUF once and reused by the
    k/v/q projection passes; wo is resident too (v1 re-streamed both).
  - emission order k-proj, v-proj, then per s-block: q-proj chunk ->
    yout(prev blk) -> attention(blk). Attention for block 0 starts as
    soon as k/v and the first q chunk are done; remaining q chunks and
    the out-projection fill PE stalls during the scalar-paced exp phase.
  - softmax: scores^T per 128-key tile -> exp on scalar engine (mask as
    per-partition bias), pairwise sums of exp tiles on DVE (bf16 2x),
    then an accumulated ones-stationary matmul folds the remaining 8-way
    sum AND the partition broadcast into one PE chain (fp32, exact).
    reciprocal on DVE over the broadcast [128,512] (v1 did a 1-partition
    reciprocal: 4us vs 0.7us here), normalization fused into PV eviction.
  - DMA split across queues: weights on the Sync queue, x loads and yT
    stores on the GpSimd queue.
  - output yT in bf16 (halves store traffic); host sums partials in fp32.
"""

import os
import numpy as np
import ml_dtypes

import concourse.bass as bass
import concourse.tile as tile
from concourse import bacc, mybir
from concourse import bass_utils

B, S, D = 2, 2048, 2048
NH, HD = 16, 128
N_CORES = 8
HPC = 4                      # heads per core
E = HPC * HD                 # 512: per-core q/k/v width
SCALE = float(HD) ** -0.5
F32 = mybir.dt.float32
BF16 = mybir.dt.bfloat16

P = 128                      # partition tile
ST = S // P                  # 16 s partition-tiles
DTI = D // P                 # 16 d partition-tiles
SB = 512                     # matmul moving-dim block / query block
NBLK = S // SB               # 4 s blocks
MASK_MIN = float(np.finfo(np.float32).min)
NPBF16 = ml_dtypes.bfloat16


def _build(has_bias: bool):
    nc = bacc.Bacc("TRN2", target_bir_lowering=False, debug=False,
                   num_devices=N_CORES)

    xTd = nc.dram_tensor("xT", [D, S], BF16, kind="ExternalInput").ap()
    wqT = nc.dram_tensor("wqT", [D, E], BF16, kind="ExternalInput").ap()
    wkT = nc.dram_tensor("wkT", [D, E], BF16, kind="ExternalInput").ap()
    wvT = nc.dram_tensor("wvT", [D, E], BF16, kind="ExternalInput").ap()
    woT = nc.dram_tensor("woT", [E, D], BF16, kind="ExternalInput").ap()
    maskT = nc.dram_tensor("maskT", [S], F32, kind="ExternalInput").ap()
    ones2 = nc.dram_tensor("ones2", [P, SB], BF16, kind="ExternalInput").ap()
    if has_bias:
        bqd = nc.dram_tensor("bq", [E], BF16, kind="ExternalInput").ap()
        bkd = nc.dram_tensor("bk", [E], BF16, kind="ExternalInput").ap()
        bvd = nc.dram_tensor("bv", [E], BF16, kind="ExternalInput").ap()
    yT = nc.dram_tensor("yT", [D, S], BF16, kind="ExternalOutput").ap()

    with tile.TileContext(nc) as tc:
        with tc.tile_pool(name="persist", bufs=1) as persist, \
             tc.tile_pool(name="xpool", bufs=1) as xpool:
            kT = [persist.tile([P, S], BF16, name=f"kT{i}", tag=f"kT{i}")
                  for i in range(HPC)]
            vv = [persist.tile([P, E], BF16, name=f"v{i}", tag=f"v{i}")
                  for i in range(ST)]
            mask_sb = persist.tile([P, ST], F32, name="mask_sb", tag="mask")
            nc.sync.dma_start(mask_sb[:, :],
                              maskT.rearrange("(t p) -> p t", p=P))
            ones_sb = persist.tile([P, P], BF16, name="ones_sb", tag="ones")
            nc.sync.dma_start(ones_sb[:, :], ones2[:, 0:P])
            if has_bias:
                ones_row = persist.tile([1, SB], BF16, name="ones_row",
                                        tag="onesr")
                nc.sync.dma_start(ones_row[:, :], ones2[0:1, :])
                ones_rp = persist.tile([1, P], BF16, name="ones_rp",
                                       tag="onesrp")
                nc.sync.dma_start(ones_rp[:, :], ones2[0:1, 0:P])
                bq_sb = persist.tile([1, E], BF16, name="bq_sb", tag="bq")
                bk_sb = persist.tile([1, E], BF16, name="bk_sb", tag="bk")
                bv_sb = persist.tile([1, E], BF16, name="bv_sb", tag="bv")
                nc.sync.dma_start(bq_sb[:, :], bqd.rearrange("(a e) -> a e", a=1))
                nc.sync.dma_start(bk_sb[:, :], bkd.rearrange("(a e) -> a e", a=1))
                nc.sync.dma_start(bv_sb[:, :], bvd.rearrange("(a e) -> a e", a=1))
            # wo tiles are persistent but DMA'd later (after the k/v weight
            # loads) so they don't hog the Sync queue at startup.
            wo_sb = [[persist.tile([P, P], BF16, name=f"wo_{dv}_{eo}",
                                   tag=f"wo_{dv}_{eo}")
                      for eo in range(DTI)] for dv in range(HPC)]

            # x resident: 16 [128, 2048] bf16 tiles, loaded chunk-major on
            # the gpsimd DMA queue so the k-projection can start on chunk 0.
            xx = [xpool.tile([P, S], BF16, name=f"x{dt}", tag=f"x{dt}")
                  for dt in range(DTI)]
            for ch in range(NBLK):
                c0 = ch * SB
                for dt in range(DTI):
                    nc.gpsimd.dma_start(xx[dt][:, c0:c0 + SB],
                                        xTd[dt * P:(dt + 1) * P, c0:c0 + SB])

            # ---------------- Phase K: k projection ----------------
            # kT[e, s] = wkT.T-slice @ x  (+ bk). dt-major emission: 4
            # interleaved psum chains consume w tiles in DMA arrival order.
            kvps = tc.tile_pool(name="ps_kv", bufs=4, space="PSUM")
            pskv = kvps.__enter__()
            with nc.named_scope("proj_k"), \
                 tc.tile_pool(name="wk", bufs=1) as wkpool:
                wk_sb = [[None] * HPC for _ in range(DTI)]
                for dt in range(DTI):
                    for et in range(HPC):
                        w_t = wkpool.tile([P, P], BF16, name=f"wk_{dt}_{et}",
                                          tag=f"wk_{dt}_{et}")
                        nc.sync.dma_start(
                            w_t[:, :],
                            wkT[dt * P:(dt + 1) * P, et * P:(et + 1) * P])
                        wk_sb[dt][et] = w_t
                for ch in range(NBLK):
                    c0 = ch * SB
                    pss = [pskv.tile([P, SB], F32, name="ps_k")
                           for _ in range(HPC)]
                    for dt in range(DTI):
                        for et in range(HPC):
                            nc.tensor.matmul(
                                pss[et][:, :], wk_sb[dt][et][:, :],
                                xx[dt][:, c0:c0 + SB],
                                start=(dt == 0),
                                stop=(dt == DTI - 1 and not has_bias))
                    for et in range(HPC):
                        if has_bias:
                            nc.tensor.matmul(
                                pss[et][:, :],
                                bk_sb[0:1, et * P:(et + 1) * P],
                                ones_row[0:1, 0:SB],
                                start=False, stop=True)
                        nc.scalar.copy(kT[et][:, c0:c0 + SB], pss[et][:, :])

            # ---------------- Phase V: v projection ----------------
            # v[s, e] = x-slice.T @ wvT (+ bv), natural layout.
            with nc.named_scope("proj_v"), \
                 tc.tile_pool(name="wv", bufs=1) as wvpool:
                psv = pskv
                wv_sb = []
                for dt in range(DTI):
                    wv_t = wvpool.tile([P, E], BF16, name=f"wv_{dt}",
                                       tag=f"wv_{dt}")
                    nc.sync.dma_start(wv_t[:, :], wvT[dt * P:(dt + 1) * P, :])
                    wv_sb.append(wv_t)
                for ch in range(NBLK):
                    c0 = ch * SB
                    pss = [psv.tile([P, E], F32, name="ps_vt")
                           for _ in range(HPC)]
                    for dt in range(DTI):
                        for sl in range(HPC):
                            nc.tensor.matmul(
                                pss[sl][:, :],
                                xx[dt][:, c0 + sl * P:c0 + (sl + 1) * P],
                                wv_sb[dt][:, :],
                                start=(dt == 0),
                                stop=(dt == DTI - 1 and not has_bias))
                    for sl in range(HPC):
                        st = ch * HPC + sl
                        if has_bias:
                            nc.tensor.matmul(
                                pss[sl][:, :], ones_rp[0:1, :],
                                bv_sb[0:1, :], start=False, stop=True)
                        nc.scalar.copy(vv[st][:, :], pss[sl][:, :])
            kvps.__exit__(None, None, None)

            # ---------------- Phase QA: q proj + attention + yout ----------
            with nc.named_scope("attn"), \
                 tc.tile_pool(name="wq", bufs=1) as wqpool, \
                 tc.tile_pool(name="qtn", bufs=2 * HPC) as qpool, \
                 tc.tile_pool(name="otn", bufs=2 * HPC) as opool, \
                 tc.tile_pool(name="expp", bufs=20) as expp, \
                 tc.tile_pool(name="prtp", bufs=10) as prtp, \
                 tc.tile_pool(name="smx", bufs=2) as smx, \
                 tc.tile_pool(name="stage", bufs=3) as stagep, \
                 tc.tile_pool(name="ps_q", bufs=1, space="PSUM") as ps_q, \
                 tc.tile_pool(name="ps_sc", bufs=4, space="PSUM") as ps_sc, \
                 tc.tile_pool(name="ps_att", bufs=1, space="PSUM") as ps_att:
                ps_y = ps_q   # q-proj and yout are time-disjoint; share banks
                # weight DMAs: wq first (needed first), then wo.
                wq_sb = [[None] * HPC for _ in range(DTI)]
                for dt in range(DTI):
                    for et in range(HPC):
                        w_t = wqpool.tile([P, P], BF16, name=f"wq_{dt}_{et}",
                                          tag=f"wq_{dt}_{et}")
                        nc.sync.dma_start(
                            w_t[:, :],
                            wqT[dt * P:(dt + 1) * P, et * P:(et + 1) * P])
                        wq_sb[dt][et] = w_t
                for dv in range(HPC):
                    for eo in range(DTI):
                        nc.sync.dma_start(
                            wo_sb[dv][eo][:, :],
                            woT[dv * P:(dv + 1) * P, eo * P:(eo + 1) * P])

                def emit_qproj(blk):
                    """q chunk blk -> 4 [128,512] bf16 tiles (SCALE folded
                    into wq on host)."""
                    c0 = blk * SB
                    qts = []
                    for et in range(HPC):
                        ps = ps_q.tile([P, SB], F32, name="ps_qt")
                        for dt in range(DTI):
                            nc.tensor.matmul(
                                ps[:, :], wq_sb[dt][et][:, :],
                                xx[dt][:, c0:c0 + SB],
                                start=(dt == 0),
                                stop=(dt == DTI - 1 and not has_bias))
                        if has_bias:
                            nc.tensor.matmul(
                                ps[:, :], bq_sb[0:1, et * P:(et + 1) * P],
                                ones_row[0:1, 0:SB], start=False, stop=True)
                        qt = qpool.tile([P, SB], BF16, name="qt")
                        nc.scalar.copy(qt[:, :], ps[:, :])
                        qts.append(qt)
                    return qts

                def emit_yout(blk, ots):
                    """partial out-projection for block blk."""
                    q0 = blk * SB
                    for eo in range(DTI):
                        yps = ps_y.tile([P, SB], F32, name="yps")
                        for dv in range(HPC):
                            nc.tensor.matmul(
                                yps[:, :], wo_sb[dv][eo][:, :],
                                ots[dv][:, :],
                                start=(dv == 0), stop=(dv == HPC - 1))
                        stg = stagep.tile([P, SB], BF16, name="stg")
                        nc.vector.tensor_copy(stg[:, :], yps[:, :])
                        nc.gpsimd.dma_start(
                            yT[eo * P:(eo + 1) * P, q0:q0 + SB], stg[:, :])

                def start_pv_r(h, ex, prt):
                    """allocate psum tiles for head h's PV and r chains."""
                    ops = ps_att.tile([P, SB], F32, name="ops")
                    rps = ps_att.tile([P, SB], F32, name="rps")
                    return {"h": h, "ex": ex, "prt": prt,
                            "ops": ops, "rps": rps}

                def step_pv_r(pend, sk):
                    """one PV matmul (+ every other step one r matmul),
                    interleaved into the next head's scores to keep PE busy
                    during the scalar-paced exp stretch."""
                    h, ex = pend["h"], pend["ex"]
                    nc.tensor.matmul(
                        pend["ops"][:, :],
                        vv[sk][:, h * P:(h + 1) * P], ex[sk][:, :],
                        start=(sk == 0), stop=(sk == ST - 1))
                    if sk % 2 == 0:
                        i = sk // 2
                        nc.tensor.matmul(
                            pend["rps"][:, :], ones_sb[:, :],
                            pend["prt"][i][:, :],
                            start=(i == 0), stop=(i == ST // 2 - 1))

                def finish_pv_r(pend, ots):
                    """reciprocal of the broadcast row-sums, normalize PV
                    output into oTn."""
                    rbc = smx.tile([P, SB], F32, name="rbc")
                    nc.vector.reciprocal(rbc[:, :], pend["rps"][:, :])
                    ot = opool.tile([P, SB], BF16, name="ot")
                    nc.vector.tensor_mul(ot[:, :], pend["ops"][:, :],
                                         rbc[:, :])
                    ots.append(ot)

                prev_ots = None
                for blk in range(NBLK):
                    q0 = blk * SB
                    qts = emit_qproj(blk)
                    if prev_ots is not None:
                        emit_yout(blk - 1, prev_ots)
                    ots = []
                    pend = None
                    for h in range(HPC):
                        ex = []
                        for sk in range(ST):
                            ps = ps_sc.tile([P, SB], F32, name="ps_sct")
                            nc.tensor.matmul(
                                ps[:, :],
                                kT[h][:, sk * P:(sk + 1) * P],
                                qts[h][:, :],
                                start=True, stop=True)
                            ext = expp.tile([P, SB], BF16, name="ext")
                            nc.scalar.activation(
                                ext[:, :], ps[:, :],
                                mybir.ActivationFunctionType.Exp,
                                bias=mask_sb[:, sk:sk + 1], scale=1.0)
                            ex.append(ext)
                            if pend is not None:
                                step_pv_r(pend, sk)
                        # pairwise partial sums of the 16 exp tiles (DVE)
                        prt = []
                        for i in range(ST // 2):
                            pt = prtp.tile([P, SB], BF16, name="prt")
                            nc.vector.tensor_add(pt[:, :],
                                                 ex[2 * i][:, :],
                                                 ex[2 * i + 1][:, :])
                            prt.append(pt)
                        if pend is not None:
                            finish_pv_r(pend, ots)
                        pend = start_pv_r(h, ex, prt)
                    # drain last head of the block: no next scores to
                    # interleave with; PE just runs the chains back-to-back.
                    for sk in range(ST):
                        step_pv_r(pend, sk)
                    finish_pv_r(pend, ots)
                    prev_ots = ots
                emit_yout(NBLK - 1, prev_ots)

    nc.compile()
    return nc


_NC_CACHE = {}


def _get_nc(has_bias: bool):
    if has_bias not in _NC_CACHE:
        _NC_CACHE[has_bias] = _build(has_bias)
    return _NC_CACHE[has_bias]


def kernel(hidden_states, attention_mask, Wq, bq, Wk, bk, Wv, bv, Wo, bo):
    hidden_states = np.asarray(hidden_states, dtype=np.float32)
    attention_mask = np.asarray(attention_mask, dtype=np.float32)
    Wq = np.asarray(Wq, dtype=np.float32)
    Wk = np.asarray(Wk, dtype=np.float32)
    Wv = np.asarray(Wv, dtype=np.float32)
    Wo = np.asarray(Wo, dtype=np.float32)
    bq = np.asarray(bq, dtype=np.float32)
    bk = np.asarray(bk, dtype=np.float32)
    bv = np.asarray(bv, dtype=np.float32)
    bo = np.asarray(bo, dtype=np.float32)

    has_bias = bool(np.any(bq) or np.any(bk) or np.any(bv))
    nc = _get_nc(has_bias)

    # Host-side sharding prep (cheap numpy work, not on the HW critical path)
    xT = [np.ascontiguousarray(hidden_states[b].T).astype(NPBF16)
          for b in range(B)]
    addmask = [np.ascontiguousarray((1.0 - attention_mask[b]) * MASK_MIN)
               for b in range(B)]
    ones2 = np.ones((P, SB), dtype=NPBF16)
    Wq_s = Wq * SCALE          # fold the 1/sqrt(hd) scale into Wq (and bq)
    in_maps = []
    for c in range(N_CORES):
        b, g = c // 4, c % 4
        sl = slice(g * E, (g + 1) * E)
        im = {
            "xT": xT[b],
            "wqT": np.ascontiguousarray(Wq_s[sl, :].T).astype(NPBF16),
            "wkT": np.ascontiguousarray(Wk[sl, :].T).astype(NPBF16),
            "wvT": np.ascontiguousarray(Wv[sl, :].T).astype(NPBF16),
            "woT": np.ascontiguousarray(Wo[:, sl].T).astype(NPBF16),
            "maskT": addmask[b],
            "ones2": ones2,
        }
        if has_bias:
            im["bq"] = np.ascontiguousarray(bq[sl] * SCALE).astype(NPBF16)
            im["bk"] = np.ascontiguousarray(bk[sl]).astype(NPBF16)
            im["bv"] = np.ascontiguousarray(bv[sl]).astype(NPBF16)
        in_maps.append(im)

    res = bass_utils.run_bass_kernel_spmd(
        nc, in_maps, core_ids=list(range(N_CORES)),
        trace=bool(int(os.environ.get("BASS_KERNEL_TRACE", "0"))))
    kernel.last_results = res

    out = np.empty((B, S, D), dtype=np.float32)
    for b in range(B):
        acc = res.results[b * 4]["yT"].astype(np.float32)
        for g in range(1, 4):
            acc += res.results[b * 4 + g]["yT"].astype(np.float32)
        out[b] = acc.T + bo
    return out


# revision 17
# speedup vs baseline: 1.6753x; 1.0994x over previous
"""LlamaAttention (B=2, S=2048, D=2048, H=16) on 8 Trainium2 NeuronCores.

Sharding: batch x head-group. Core c handles batch b = c // 4 and head group
g = c % 4 (4 heads of 128 dims each -> a 512-wide slice of q/k/v space).
Each core computes q/k/v projections for its slice, attention for its 4
heads, and a partial out-projection (contracting only its 512 dv dims).
Host sums the 4 partials per batch and adds the output bias.

v3 design (v1 fp32r ~812us, v2 bf16 ~533us):
  - all matmul inputs bf16 (PE issues a [128,128]x[128,512] matmul every
    216ns back-to-back; fp32r took 466ns). PSUM accumulation stays fp32.
  - x and wo resident in SBUF; weights streamed on two DMA queues
    (sync + vector) so the k-projection isn't DMA-starved at startup.
  - attention emission keeps the PE queue dense: the q-projection of the
    next block and the out-projection of the previous block are chopped
    into per-matmul closures and interleaved 2-per-score-slot into the
    scores/PV stream, filling the stalls while the scalar engine runs exp.
  - exp fused 4 key-tiles wide: scores land in a 4-bank [128,2048] PSUM
    tile, one ACTIVATE evicts ~9.1us/head-block of scalar work down from
    ~15.8us (352-elem fixed overhead + semaphore amortized 4x). Only
    valid when the attention mask is all-ones (bias is per-partition);
    a general-mask variant with per-tile bias is built lazily.
  - softmax denominator: 15 bf16 pairwise/tree adds on DVE, one
    ones-stationary matmul (fp32 PSUM) folds the final partition
    reduction AND the broadcast, then reciprocal_approx_fast (~5x faster
    than DVE reciprocal, 51 ULP -- noise floor here is bf16 at ~2^-9).
    Normalization fused into the PV-psum eviction.
  - output yT in bf16; host sums the 4 partials per batch in fp32.
"""

import os
import numpy as np
import ml_dtypes

import concourse.bass as bass
import concourse.tile as tile
from concourse import bacc, mybir
from concourse import bass_utils

B, S, D = 2, 2048, 2048
NH, HD = 16, 128
N_CORES = 8
HPC = 4                      # heads per core
E = HPC * HD                 # 512: per-core q/k/v width
SCALE = float(HD) ** -0.5
F32 = mybir.dt.float32
BF16 = mybir.dt.bfloat16

P = 128                      # partition tile
ST = S // P                  # 16 s partition-tiles
DTI = 16                     # d partition-tiles
SB = 512                     # matmul moving-dim block / query block
NBLK = S // SB               # 4 s blocks
GRP = 4                      # key tiles fused per exp ACTIVATE
NG = ST // GRP               # 4 score groups per head-block
MASK_MIN = float(np.finfo(np.float32).min)
NPBF16 = ml_dtypes.bfloat16


def _build(has_bias: bool, mask_trivial: bool):
    nc = bacc.Bacc("TRN2", target_bir_lowering=False, debug=False,
                   num_devices=N_CORES)

    xTd = nc.dram_tensor("xT", [D, S], BF16, kind="ExternalInput").ap()
    wqT = nc.dram_tensor("wqT", [D, E], BF16, kind="ExternalInput").ap()
    wkT = nc.dram_tensor("wkT", [D, E], BF16, kind="ExternalInput").ap()
    wvT = nc.dram_tensor("wvT", [D, E], BF16, kind="ExternalInput").ap()
    woT = nc.dram_tensor("woT", [E, D], BF16, kind="ExternalInput").ap()
    maskT = nc.dram_tensor("maskT", [S], F32, kind="ExternalInput").ap()
    ones2 = nc.dram_tensor("ones2", [P, SB], BF16, kind="ExternalInput").ap()
    if has_bias:
        bqd = nc.dram_tensor("bq", [E], BF16, kind="ExternalInput").ap()
        bkd = nc.dram_tensor("bk", [E], BF16, kind="ExternalInput").ap()
        bvd = nc.dram_tensor("bv", [E], BF16, kind="ExternalInput").ap()
    yT = nc.dram_tensor("yT", [D, S], BF16, kind="ExternalOutput").ap()

    # alternate weight-tile DMAs between two queues
    wq_queues = [None, None]

    with tile.TileContext(nc) as tc:
        with tc.tile_pool(name="persist", bufs=1) as persist, \
             tc.tile_pool(name="xpool", bufs=1) as xpool:
            wq_queues[0] = nc.sync
            wq_queues[1] = nc.scalar
            kT = [persist.tile([P, S], BF16, name=f"kT{i}", tag=f"kT{i}")
                  for i in range(HPC)]
            vv = [persist.tile([P, E], BF16, name=f"v{i}", tag=f"v{i}")
                  for i in range(ST)]
            mask_sb = persist.tile([P, ST], F32, name="mask_sb", tag="mask")
            nc.sync.dma_start(mask_sb[:, :],
                              maskT.rearrange("(t p) -> p t", p=P))
            ones_sb = persist.tile([P, P], BF16, name="ones_sb", tag="ones")
            nc.sync.dma_start(ones_sb[:, :], ones2[:, 0:P])
            if has_bias:
                ones_row = persist.tile([1, SB], BF16, name="ones_row",
                                        tag="onesr")
                nc.sync.dma_start(ones_row[:, :], ones2[0:1, :])
                ones_rp = persist.tile([1, P], BF16, name="ones_rp",
                                       tag="onesrp")
                nc.sync.dma_start(ones_rp[:, :], ones2[0:1, 0:P])
                bq_sb = persist.tile([1, E], BF16, name="bq_sb", tag="bq")
                bk_sb = persist.tile([1, E], BF16, name="bk_sb", tag="bk")
                bv_sb = persist.tile([1, E], BF16, name="bv_sb", tag="bv")
                nc.sync.dma_start(bq_sb[:, :], bqd.rearrange("(a e) -> a e", a=1))
                nc.sync.dma_start(bk_sb[:, :], bkd.rearrange("(a e) -> a e", a=1))
                nc.sync.dma_start(bv_sb[:, :], bvd.rearrange("(a e) -> a e", a=1))
            wo_sb = [[persist.tile([P, P], BF16, name=f"wo_{dv}_{eo}",
                                   tag=f"wo_{dv}_{eo}")
                      for eo in range(DTI)] for dv in range(HPC)]

            # x resident, loaded chunk-major on the gpsimd DMA queue.
            xx = [xpool.tile([P, S], BF16, name=f"x{dt}", tag=f"x{dt}")
                  for dt in range(DTI)]
            for ch in range(NBLK):
                c0 = ch * SB
                for dt in range(DTI):
                    nc.gpsimd.dma_start(xx[dt][:, c0:c0 + SB],
                                        xTd[dt * P:(dt + 1) * P, c0:c0 + SB])

            # ---------------- Phase K: k projection ----------------
            kvps = tc.tile_pool(name="ps_kv", bufs=4, space="PSUM")
            pskv = kvps.__enter__()
            with nc.named_scope("proj_k"), \
                 tc.tile_pool(name="wk", bufs=1) as wkpool:
                wk_sb = [[None] * HPC for _ in range(DTI)]
                for dt in range(DTI):
                    for et in range(HPC):
                        w_t = wkpool.tile([P, P], BF16, name=f"wk_{dt}_{et}",
                                          tag=f"wk_{dt}_{et}")
                        wq_queues[(dt * HPC + et) % 2].dma_start(
                            w_t[:, :],
                            wkT[dt * P:(dt + 1) * P, et * P:(et + 1) * P])
                        wk_sb[dt][et] = w_t
                for ch in range(NBLK):
                    c0 = ch * SB
                    pss = [pskv.tile([P, SB], F32, name="ps_k")
                           for _ in range(HPC)]
                    for dt in range(DTI):
                        for et in range(HPC):
                            nc.tensor.matmul(
                                pss[et][:, :], wk_sb[dt][et][:, :],
                                xx[dt][:, c0:c0 + SB],
                                start=(dt == 0),
                                stop=(dt == DTI - 1 and not has_bias))
                    for et in range(HPC):
                        if has_bias:
                            nc.tensor.matmul(
                                pss[et][:, :],
                                bk_sb[0:1, et * P:(et + 1) * P],
                                ones_row[0:1, 0:SB],
                                start=False, stop=True)
                        nc.scalar.copy(kT[et][:, c0:c0 + SB], pss[et][:, :])

            # ---------------- Phase V: v projection ----------------
            with nc.named_scope("proj_v"), \
                 tc.tile_pool(name="wv", bufs=1) as wvpool:
                wv_sb = []
                for dt in range(DTI):
                    wv_t = wvpool.tile([P, E], BF16, name=f"wv_{dt}",
                                       tag=f"wv_{dt}")
                    nc.sync.dma_start(
                        wv_t[:, :], wvT[dt * P:(dt + 1) * P, :])
                    wv_sb.append(wv_t)
                for ch in range(NBLK):
                    c0 = ch * SB
                    pss = [pskv.tile([P, E], F32, name="ps_vt")
                           for _ in range(HPC)]
                    for dt in range(DTI):
                        for sl in range(HPC):
                            nc.tensor.matmul(
                                pss[sl][:, :],
                                xx[dt][:, c0 + sl * P:c0 + (sl + 1) * P],
                                wv_sb[dt][:, :],
                                start=(dt == 0),
                                stop=(dt == DTI - 1 and not has_bias))
                    for sl in range(HPC):
                        st = ch * HPC + sl
                        if has_bias:
                            nc.tensor.matmul(
                                pss[sl][:, :], ones_rp[0:1, :],
                                bv_sb[0:1, :], start=False, stop=True)
                        nc.scalar.copy(vv[st][:, :], pss[sl][:, :])
            kvps.__exit__(None, None, None)

            # ---------------- Phase QA: q proj + attention + yout ----------
            scw = 2 * SB if not mask_trivial else GRP * SB
            with nc.named_scope("attn"), \
                 tc.tile_pool(name="wq", bufs=1) as wqpool, \
                 tc.tile_pool(name="qtn", bufs=2 * HPC) as qpool, \
                 tc.tile_pool(name="otn", bufs=2 * HPC) as opool, \
                 tc.tile_pool(name="expp", bufs=9 * (2048 // scw)) as expp, \
                 tc.tile_pool(name="prtp", bufs=12) as prtp, \
                 tc.tile_pool(name="smx", bufs=2) as smx, \
                 tc.tile_pool(name="stage", bufs=3) as stagep, \
                 tc.tile_pool(name="ps_q", bufs=1, space="PSUM") as ps_q, \
                 tc.tile_pool(name="ps_sc", bufs=4 * SB // scw,
                              space="PSUM") as ps_sc, \
                 tc.tile_pool(name="ps_att", bufs=1, space="PSUM") as ps_att:
                ps_y = ps_q   # q-proj and yout share banks (distinct tags)
                wq_sb = [[None] * HPC for _ in range(DTI)]
                for dt in range(DTI):
                    for et in range(HPC):
                        w_t = wqpool.tile([P, P], BF16, name=f"wq_{dt}_{et}",
                                          tag=f"wq_{dt}_{et}")
                        nc.sync.dma_start(
                            w_t[:, :],
                            wqT[dt * P:(dt + 1) * P, et * P:(et + 1) * P])
                        wq_sb[dt][et] = w_t
                for dv in range(HPC):
                    for eo in range(DTI):
                        nc.sync.dma_start(
                            wo_sb[dv][eo][:, :],
                            woT[dv * P:(dv + 1) * P, eo * P:(eo + 1) * P])

                def qproj_closures(blk, sink):
                    """per-matmul closures for q chunk blk; appends the 4
                    evicted bf16 q tiles to sink."""
                    c0 = blk * SB
                    cls = []
                    for et in range(HPC):
                        hold = {}
                        for dt in range(DTI):
                            def cl(et=et, dt=dt, c0=c0, hold=hold):
                                if dt == 0:
                                    hold["ps"] = ps_q.tile([P, SB], F32,
                                                           name="ps_qt")
                                ps = hold["ps"]
                                nc.tensor.matmul(
                                    ps[:, :], wq_sb[dt][et][:, :],
                                    xx[dt][:, c0:c0 + SB],
                                    start=(dt == 0),
                                    stop=(dt == DTI - 1 and not has_bias))
                                if dt == DTI - 1:
                                    if has_bias:
                                        nc.tensor.matmul(
                                            ps[:, :],
                                            bq_sb[0:1, et * P:(et + 1) * P],
                                            ones_row[0:1, 0:SB],
                                            start=False, stop=True)
                                    qt = qpool.tile([P, SB], BF16, name="qt")
                                    nc.scalar.copy(qt[:, :], ps[:, :])
                                    sink.append(qt)
                            cls.append(cl)
                    return cls

                def yout_closures(blk, ots):
                    """per-matmul closures for the partial out-projection."""
                    q0 = blk * SB
                    cls = []
                    for eo in range(DTI):
                        hold = {}
                        for dv in range(HPC):
                            def cl(eo=eo, dv=dv, q0=q0, hold=hold, ots=ots):
                                if dv == 0:
                                    hold["ps"] = ps_y.tile([P, SB], F32,
                                                           name="yps")
                                yps = hold["ps"]
                                nc.tensor.matmul(
                                    yps[:, :], wo_sb[dv][eo][:, :],
                                    ots[dv][:, :],
                                    start=(dv == 0), stop=(dv == HPC - 1))
                                if dv == HPC - 1:
                                    stg = stagep.tile([P, SB], BF16,
                                                      name="stg")
                                    nc.vector.tensor_copy(stg[:, :],
                                                          yps[:, :])
                                    nc.gpsimd.dma_start(
                                        yT[eo * P:(eo + 1) * P, q0:q0 + SB],
                                        stg[:, :])
                            cls.append(cl)
                    return cls

                def start_pv_r(h, exs):
                    ops = ps_att.tile([P, SB], F32, name="ops")
                    rps = ps_att.tile([P, SB], F32, name="rps")
                    return {"h": h, "exs": exs, "ops": ops, "rps": rps,
                            "racc": None}

                def step_pv(pend, sk):
                    h = pend["h"]
                    big, off = pend["exs"][sk * SB // scw], (sk * SB) % scw
                    nc.tensor.matmul(
                        pend["ops"][:, :],
                        vv[sk][:, h * P:(h + 1) * P],
                        big[:, off:off + SB],
                        start=(sk == 0), stop=(sk == ST - 1))

                def emit_rsum(pend):
                    """pairwise/tree bf16 adds on DVE down to one [128,512]
                    tile, then a single ones-stationary matmul for the
                    cross-partition sum + broadcast."""
                    lvl = []
                    for exb in pend["exs"]:
                        for o in range(0, scw, 2 * SB):
                            pt = prtp.tile([P, SB], BF16, name="prt")
                            nc.vector.tensor_add(pt[:, :],
                                                 exb[:, o:o + SB],
                                                 exb[:, o + SB:o + 2 * SB])
                            lvl.append(pt)
                    while len(lvl) > 1:
                        nxt = []
                        for i in range(0, len(lvl), 2):
                            pt = prtp.tile([P, SB], BF16, name="prt")
                            nc.vector.tensor_add(pt[:, :], lvl[i][:, :],
                                                 lvl[i + 1][:, :])
                            nxt.append(pt)
                        lvl = nxt
                    pend["racc"] = lvl[0]

                def finish_pv_r(pend, ots):
                    # ones-matmul emitted here (not at emit_rsum time) so the
                    # PE queue never waits on the DVE add tree: by now the
                    # tree ran during the next head's scores.
                    nc.tensor.matmul(pend["rps"][:, :], ones_sb[:, :],
                                     pend["racc"][:, :], start=True, stop=True)
                    rbc = smx.tile([P, SB], F32, name="rbc")
                    nc.vector.reciprocal_approx_fast(rbc[:, :],
                                                     pend["rps"][:, :])
                    ot = opool.tile([P, SB], BF16, name="ot")
                    nc.vector.tensor_mul(ot[:, :], pend["ops"][:, :],
                                         rbc[:, :])
                    ots.append(ot)

                # q chunk 0 emitted plain (nothing to interleave with yet)
                qts = []
                for cl in qproj_closures(0, qts):
                    cl()

                prev_ots = None
                next_qts = None
                for blk in range(NBLK):
                    fillers = []
                    if prev_ots is not None:
                        fillers += yout_closures(blk - 1, prev_ots)
                    next_qts = []
                    if blk + 1 < NBLK:
                        fillers += qproj_closures(blk + 1, next_qts)
                    fidx = [0]

                    def fill(n):
                        for _ in range(n):
                            if fidx[0] < len(fillers):
                                fillers[fidx[0]]()
                                fidx[0] += 1

                    ots = []
                    pend = None
                    for h in range(HPC):
                        exs = []
                        for g in range(ST * SB // scw):
                            ps = ps_sc.tile([P, scw], F32, name="ps_sct")
                            for j in range(scw // SB):
                                sk = g * (scw // SB) + j
                                nc.tensor.matmul(
                                    ps[:, j * SB:(j + 1) * SB],
                                    kT[h][:, sk * P:(sk + 1) * P],
                                    qts[h][:, :],
                                    start=True, stop=True)
                                if pend is not None:
                                    step_pv(pend, sk)
                                fill(2)
                            ext = expp.tile([P, scw], BF16, name="ext")
                            if mask_trivial:
                                nc.scalar.activation(
                                    ext[:, :], ps[:, :],
                                    mybir.ActivationFunctionType.Exp,
                                    bias=0.0, scale=1.0)
                            else:
                                for j in range(scw // SB):
                                    sk = g * (scw // SB) + j
                                    nc.scalar.activation(
                                        ext[:, j * SB:(j + 1) * SB],
                                        ps[:, j * SB:(j + 1) * SB],
                                        mybir.ActivationFunctionType.Exp,
                                        bias=mask_sb[:, sk:sk + 1], scale=1.0)
                            exs.append(ext)
                        if pend is not None:
                            finish_pv_r(pend, ots)
                        pend = start_pv_r(h, exs)
                        emit_rsum(pend)
                    # drain last head of the block
                    for sk in range(ST):
                        step_pv(pend, sk)
                        fill(2)
                    fill(len(fillers))
                    finish_pv_r(pend, ots)
                    prev_ots = ots
                    qts = next_qts
                for cl in yout_closures(NBLK - 1, prev_ots):
                    cl()

    nc.compile()
    return nc


_NC_CACHE = {}


def _get_nc(has_bias: bool, mask_trivial: bool):
    key = (has_bias, mask_trivial)
    if key not in _NC_CACHE:
        _NC_CACHE[key] = _build(has_bias, mask_trivial)
    return _NC_CACHE[key]


def kernel(hidden_states, attention_mask, Wq, bq, Wk, bk, Wv, bv, Wo, bo):
    hidden_states = np.asarray(hidden_states, dtype=np.float32)
    attention_mask = np.asarray(attention_mask, dtype=np.float32)
    Wq = np.asarray(Wq, dtype=np.float32)
    Wk = np.asarray(Wk, dtype=np.float32)
    Wv = np.asarray(Wv, dtype=np.float32)
    Wo = np.asarray(Wo, dtype=np.float32)
    bq = np.asarray(bq, dtype=np.float32)
    bk = np.asarray(bk, dtype=np.float32)
    bv = np.asarray(bv, dtype=np.float32)
    bo = np.asarray(bo, dtype=np.float32)

    has_bias = bool(np.any(bq) or np.any(bk) or np.any(bv))
    mask_trivial = bool(np.all(attention_mask == 1.0))
    nc = _get_nc(has_bias, mask_trivial)

    # Host-side sharding prep (cheap numpy work, not on the HW critical path)
    xT = [np.ascontiguousarray(hidden_states[b].T).astype(NPBF16)
          for b in range(B)]
    addmask = [np.ascontiguousarray((1.0 - attention_mask[b]) * MASK_MIN)
               for b in range(B)]
    ones2 = np.ones((P, SB), dtype=NPBF16)
    Wq_s = Wq * SCALE          # fold the 1/sqrt(hd) scale into Wq (and bq)
    in_maps = []
    for c in range(N_CORES):
        b, g = c // 4, c % 4
        sl = slice(g * E, (g + 1) * E)
        im = {
            "xT": xT[b],
            "wqT": np.ascontiguousarray(Wq_s[sl, :].T).astype(NPBF16),
            "wkT": np.ascontiguousarray(Wk[sl, :].T).astype(NPBF16),
            "wvT": np.ascontiguousarray(Wv[sl, :].T).astype(NPBF16),
            "woT": np.ascontiguousarray(Wo[:, sl].T).astype(NPBF16),
            "maskT": addmask[b],
            "ones2": ones2,
        }
        if has_bias:
            im["bq"] = np.ascontiguousarray(bq[sl] * SCALE).astype(NPBF16)
            im["bk"] = np.ascontiguousarray(bk[sl]).astype(NPBF16)
            im["bv"] = np.ascontiguousarray(bv[sl]).astype(NPBF16)
        in_maps.append(im)

    res = bass_utils.run_bass_kernel_spmd(
        nc, in_maps, core_ids=list(range(N_CORES)),
        trace=bool(int(os.environ.get("BASS_KERNEL_TRACE", "0"))))
    kernel.last_results = res

    out = np.empty((B, S, D), dtype=np.float32)
    for b in range(B):
        acc = res.results[b * 4]["yT"].astype(np.float32)
        for g in range(1, 4):
            acc += res.results[b * 4 + g]["yT"].astype(np.float32)
        out[b] = acc.T + bo
    return out


# revision 22
# speedup vs baseline: 1.7796x; 1.0622x over previous
"""LlamaAttention (B=2, S=2048, D=2048, H=16) on 8 Trainium2 NeuronCores.

Sharding: batch x head-group. Core c handles batch b = c // 4 and head group
g = c % 4 (4 heads of 128 dims each -> a 512-wide slice of q/k/v space).
Each core computes q/k/v projections for its slice, attention for its 4
heads, and a partial out-projection (contracting only its 512 dv dims).
Host sums the 4 partials per batch and adds the output bias.

v4 design (v1 fp32r ~812us, v2 bf16 ~533us, v3 ~485us):
  - all matmul inputs bf16 (PE issues a [128,128]x[128,512] matmul every
    216ns back-to-back; fp32r took 466ns). PSUM accumulation stays fp32.
  - x and wo resident in SBUF. Weight loads use wide descriptors
    ([128,512] / [128,2048] tiles, stationary operands sliced out of
    them) -- the ~600ns fixed cost per DMA_DIRECT2D dominated the v3
    startup; wk split across the sync+scalar queues, x on gpsimd.
  - the PE queue is kept dense through the scalar-paced exp stretches by
    a single global filler deque: v-projection chunks 2-3, q-projection
    chunks 1-3 and every out-projection block are chopped into
    per-matmul closures and popped 2 per score-slot. The PV/r
    accumulation of head h is interleaved into head h+1's score slots
    and carries across block boundaries.
  - exp fused 4 key-tiles wide: scores land in a 4-bank [128,2048] PSUM
    tile, one ACTIVATE per group (amortizes the 352-elem fixed overhead
    and the per-instruction semaphore 4x). Only valid with an all-ones
    attention mask (ACT bias is per-partition); a general-mask variant
    with per-tile bias is built lazily on first use.
  - softmax denominator: 15 bf16 pairwise/tree adds on DVE, one
    ones-stationary matmul folds the final partition reduction AND the
    broadcast (fp32 PSUM), reciprocal_approx_fast (~0.7us vs 3.4us for
    DVE reciprocal; 51-ULP error is far below the bf16 noise floor),
    normalization fused into the PV-psum eviction.
  - output yT in bf16; host sums the 4 partials per batch in fp32.
"""

import os
import numpy as np
import ml_dtypes

import concourse.bass as bass
import concourse.tile as tile
from concourse import bacc, mybir
from concourse import bass_utils

B, S, D = 2, 2048, 2048
NH, HD = 16, 128
N_CORES = 8
HPC = 4                      # heads per core
E = HPC * HD                 # 512: per-core q/k/v width
SCALE = float(HD) ** -0.5
F32 = mybir.dt.float32
BF16 = mybir.dt.bfloat16

P = 128                      # partition tile
ST = S // P                  # 16 s partition-tiles
DTI = 16                     # d partition-tiles
SB = 512                     # matmul moving-dim block / query block
NBLK = S // SB               # 4 s blocks
MASK_MIN = float(np.finfo(np.float32).min)
NPBF16 = ml_dtypes.bfloat16


def _build(has_bias: bool, mask_trivial: bool):
    nc = bacc.Bacc("TRN2", target_bir_lowering=False, debug=False,
                   num_devices=N_CORES)

    xTd = nc.dram_tensor("xT", [D, S], BF16, kind="ExternalInput").ap()
    wqT = nc.dram_tensor("wqT", [D, E], BF16, kind="ExternalInput").ap()
    wkT = nc.dram_tensor("wkT", [D, E], BF16, kind="ExternalInput").ap()
    wvT = nc.dram_tensor("wvT", [D, E], BF16, kind="ExternalInput").ap()
    woT = nc.dram_tensor("woT", [E, D], BF16, kind="ExternalInput").ap()
    maskT = nc.dram_tensor("maskT", [S], F32, kind="ExternalInput").ap()
    ones2 = nc.dram_tensor("ones2", [P, SB], BF16, kind="ExternalInput").ap()
    if has_bias:
        bqd = nc.dram_tensor("bq", [E], BF16, kind="ExternalInput").ap()
        bkd = nc.dram_tensor("bk", [E], BF16, kind="ExternalInput").ap()
        bvd = nc.dram_tensor("bv", [E], BF16, kind="ExternalInput").ap()
    yT = nc.dram_tensor("yT", [D, S], BF16, kind="ExternalOutput").ap()

    scw = 4 * SB if mask_trivial else 2 * SB   # exp-fusion width
    SPG = scw // SB                            # key tiles per score group
    NG = ST // SPG                             # groups per head-block

    with tile.TileContext(nc) as tc:
        with tc.tile_pool(name="persist", bufs=1) as persist, \
             tc.tile_pool(name="xpool", bufs=1) as xpool:
            kT = [persist.tile([P, S], BF16, name=f"kT{i}", tag=f"kT{i}")
                  for i in range(HPC)]
            vv = [persist.tile([P, E], BF16, name=f"v{i}", tag=f"v{i}")
                  for i in range(ST)]
            mask_sb = persist.tile([P, ST], F32, name="mask_sb", tag="mask")
            nc.sync.dma_start(mask_sb[:, :],
                              maskT.rearrange("(t p) -> p t", p=P))
            ones_sb = persist.tile([P, P], BF16, name="ones_sb", tag="ones")
            nc.sync.dma_start(ones_sb[:, :], ones2[:, 0:P])
            if has_bias:
                ones_row = persist.tile([1, SB], BF16, name="ones_row",
                                        tag="onesr")
                nc.sync.dma_start(ones_row[:, :], ones2[0:1, :])
                ones_rp = persist.tile([1, P], BF16, name="ones_rp",
                                       tag="onesrp")
                nc.sync.dma_start(ones_rp[:, :], ones2[0:1, 0:P])
                bq_sb = persist.tile([1, E], BF16, name="bq_sb", tag="bq")
                bk_sb = persist.tile([1, E], BF16, name="bk_sb", tag="bk")
                bv_sb = persist.tile([1, E], BF16, name="bv_sb", tag="bv")
                nc.sync.dma_start(bq_sb[:, :], bqd.rearrange("(a e) -> a e", a=1))
                nc.sync.dma_start(bk_sb[:, :], bkd.rearrange("(a e) -> a e", a=1))
                nc.sync.dma_start(bv_sb[:, :], bvd.rearrange("(a e) -> a e", a=1))
            # wo resident: 4 wide tiles [128, 2048], sliced per eo later.
            wo_sb = [persist.tile([P, D], BF16, name=f"wo_{dv}",
                                  tag=f"wo_{dv}") for dv in range(HPC)]

            # x resident, chunk 0 first (k-projection needs it immediately).
            xx = [xpool.tile([P, S], BF16, name=f"x{dt}", tag=f"x{dt}")
                  for dt in range(DTI)]
            for dt in range(DTI):
                nc.gpsimd.dma_start(xx[dt][:, 0:SB],
                                    xTd[dt * P:(dt + 1) * P, 0:SB])
            for dt in range(DTI):
                nc.gpsimd.dma_start(xx[dt][:, SB:S],
                                    xTd[dt * P:(dt + 1) * P, SB:S])

            # ---------------- Phase K: k projection ----------------
            # weight loads use wide [128,512] descriptors; stationary
            # operands are sliced out of them. wk split sync/scalar queues.
            kvps = tc.tile_pool(name="ps_kv", bufs=4, space="PSUM")
            pskv = kvps.__enter__()
            wkp = tc.tile_pool(name="wkp", bufs=1)
            wkpool = wkp.__enter__()
            wk_sb = []
            for dt in range(DTI):
                w_t = wkpool.tile([P, E], BF16, name=f"wk_{dt}",
                                  tag=f"wk_{dt}")
                q = nc.sync if dt % 2 == 0 else nc.scalar
                q.dma_start(w_t[:, :], wkT[dt * P:(dt + 1) * P, :])
                wk_sb.append(w_t)
            with nc.named_scope("proj_k"):
                for ch in range(NBLK):
                    c0 = ch * SB
                    pss = [pskv.tile([P, SB], F32, name="ps_k")
                           for _ in range(HPC)]
                    for dt in range(DTI):
                        for et in range(HPC):
                            nc.tensor.matmul(
                                pss[et][:, :],
                                wk_sb[dt][:, et * P:(et + 1) * P],
                                xx[dt][:, c0:c0 + SB],
                                start=(dt == 0),
                                stop=(dt == DTI - 1 and not has_bias))
                    for et in range(HPC):
                        if has_bias:
                            nc.tensor.matmul(
                                pss[et][:, :],
                                bk_sb[0:1, et * P:(et + 1) * P],
                                ones_row[0:1, 0:SB],
                                start=False, stop=True)
                        nc.scalar.copy(kT[et][:, c0:c0 + SB], pss[et][:, :])
            wkp.__exit__(None, None, None)

            # ---------------- Phase V: v projection ----------------
            wvp = tc.tile_pool(name="wvp", bufs=1)
            wvpool = wvp.__enter__()
            wv_sb = []
            for dt in range(DTI):
                w_t = wvpool.tile([P, E], BF16, name=f"wv_{dt}",
                                  tag=f"wv_{dt}")
                nc.sync.dma_start(w_t[:, :], wvT[dt * P:(dt + 1) * P, :])
                wv_sb.append(w_t)
            with nc.named_scope("proj_v"):
                for ch in range(NBLK):
                    c0 = ch * SB
                    pss = [pskv.tile([P, E], F32, name="ps_k")
                           for _ in range(HPC)]
                    for dt in range(DTI):
                        for sl in range(HPC):
                            nc.tensor.matmul(
                                pss[sl][:, :],
                                xx[dt][:, c0 + sl * P:c0 + (sl + 1) * P],
                                wv_sb[dt][:, :],
                                start=(dt == 0),
                                stop=(dt == DTI - 1 and not has_bias))
                    for sl in range(HPC):
                        st = ch * HPC + sl
                        if has_bias:
                            nc.tensor.matmul(
                                pss[sl][:, :], ones_rp[0:1, :],
                                bv_sb[0:1, :], start=False, stop=True)
                        nc.scalar.copy(vv[st][:, :], pss[sl][:, :])
            wvp.__exit__(None, None, None)
            kvps.__exit__(None, None, None)

            # ---------------- Phase QA: q proj + attention + yout ----------
            with nc.named_scope("attn"), \
                 tc.tile_pool(name="wqp", bufs=1) as wqpool, \
                 tc.tile_pool(name="qtn", bufs=2 * HPC) as qpool, \
                 tc.tile_pool(name="otn", bufs=2 * HPC) as opool, \
                 tc.tile_pool(name="expp", bufs=9 * (4 * SB // scw)) as expp, \
                 tc.tile_pool(name="prtp", bufs=10) as prtp, \
                 tc.tile_pool(name="smx", bufs=1) as smx, \
                 tc.tile_pool(name="stage", bufs=2) as stagep, \
                 tc.tile_pool(name="ps_q", bufs=1, space="PSUM") as ps_q, \
                 tc.tile_pool(name="ps_sc", bufs=4 * SB // scw,
                              space="PSUM") as ps_sc, \
                 tc.tile_pool(name="ps_att", bufs=1, space="PSUM") as ps_att:
                wq_sb = []
                for dt in range(DTI):
                    w_t = wqpool.tile([P, E], BF16, name=f"wq_{dt}",
                                      tag=f"wq_{dt}")
                    nc.sync.dma_start(w_t[:, :], wqT[dt * P:(dt + 1) * P, :])
                    wq_sb.append(w_t)
                for dv in range(HPC):
                    nc.sync.dma_start(wo_sb[dv][:, :],
                                      woT[dv * P:(dv + 1) * P, :])

                def qproj_closures(blk, sink):
                    c0 = blk * SB
                    cls = []
                    for et in range(HPC):
                        hold = {}
                        for dt in range(DTI):
                            def cl(et=et, dt=dt, c0=c0, hold=hold, sink=sink):
                                if dt == 0:
                                    hold["ps"] = ps_q.tile([P, SB], F32,
                                                           name="ps_qt")
                                ps = hold["ps"]
                                nc.tensor.matmul(
                                    ps[:, :],
                                    wq_sb[dt][:, et * P:(et + 1) * P],
                                    xx[dt][:, c0:c0 + SB],
                                    start=(dt == 0),
                                    stop=(dt == DTI - 1 and not has_bias))
                                if dt == DTI - 1:
                                    if has_bias:
                                        nc.tensor.matmul(
                                            ps[:, :],
                                            bq_sb[0:1, et * P:(et + 1) * P],
                                            ones_row[0:1, 0:SB],
                                            start=False, stop=True)
                                    qt = qpool.tile([P, SB], BF16, name="qt")
                                    nc.scalar.copy(qt[:, :], ps[:, :])
                                    sink.append(qt)
                            cls.append(cl)
                    return cls

                def yout_closures(blk, ots):
                    q0 = blk * SB
                    cls = []
                    for eo in range(DTI):
                        hold = {}
                        for dv in range(HPC):
                            def cl(eo=eo, dv=dv, q0=q0, hold=hold, ots=ots):
                                if dv == 0:
                                    hold["ps"] = ps_q.tile([P, SB], F32,
                                                           name="yps")
                                yps = hold["ps"]
                                nc.tensor.matmul(
                                    yps[:, :],
                                    wo_sb[dv][:, eo * P:(eo + 1) * P],
                                    ots[dv][:, :],
                                    start=(dv == 0), stop=(dv == HPC - 1))
                                if dv == HPC - 1:
                                    stg = stagep.tile([P, SB], BF16,
                                                      name="stg")
                                    nc.vector.tensor_copy(stg[:, :],
                                                          yps[:, :])
                                    nc.gpsimd.dma_start(
                                        yT[eo * P:(eo + 1) * P, q0:q0 + SB],
                                        stg[:, :])
                            cls.append(cl)
                    return cls

                fillers = []
                fidx = [0]

                def fill(n):
                    while n > 0 and fidx[0] < len(fillers):
                        fillers[fidx[0]]()
                        fidx[0] += 1
                        n -= 1

                def start_pv_r(blk, h, exs, qt, ots):
                    ops = ps_att.tile([P, SB], F32, name="ops")
                    rps = ps_att.tile([P, SB], F32, name="rps")
                    return {"blk": blk, "h": h, "exs": exs, "qt": qt,
                            "ops": ops, "rps": rps, "racc": None, "ots": ots}

                def step_pv(pend, sk):
                    h = pend["h"]
                    big, off = pend["exs"][sk * SB // scw], (sk * SB) % scw
                    nc.tensor.matmul(
                        pend["ops"][:, :],
                        vv[sk][:, h * P:(h + 1) * P],
                        big[:, off:off + SB],
                        start=(sk == 0), stop=(sk == ST - 1))

                def emit_rsum(pend):
                    lvl = []
                    for exb in pend["exs"]:
                        for o in range(0, scw, 2 * SB):
                            pt = prtp.tile([P, SB], BF16, name="prt")
                            nc.vector.tensor_add(pt[:, :],
                                                 exb[:, o:o + SB],
                                                 exb[:, o + SB:o + 2 * SB])
                            lvl.append(pt)
                    while len(lvl) > 1:
                        nxt = []
                        for i in range(0, len(lvl), 2):
                            pt = prtp.tile([P, SB], BF16, name="prt")
                            nc.vector.tensor_add(pt[:, :], lvl[i][:, :],
                                                 lvl[i + 1][:, :])
                            nxt.append(pt)
                        lvl = nxt
                    pend["racc"] = lvl[0]

                def finish_pv_r(pend):
                    # ones-matmul emitted at finish time so the PE queue never
                    # waits on the DVE add tree (it ran during the next head's
                    # scores).
                    nc.tensor.matmul(pend["rps"][:, :], ones_sb[:, :],
                                     pend["racc"][:, :], start=True, stop=True)
                    rbc = smx.tile([P, SB], F32, name="rbc")
                    nc.vector.reciprocal_approx_fast(rbc[:, :],
                                                     pend["rps"][:, :])
                    ot = opool.tile([P, SB], BF16, name="ot")
                    nc.vector.tensor_mul(ot[:, :], pend["ops"][:, :],
                                         rbc[:, :])
                    pend["ots"].append(ot)
                    if pend["h"] == HPC - 1:
                        fillers.extend(yout_closures(pend["blk"], pend["ots"]))

                # q chunk 0 emitted plain; q1 seeds the deque
                qsinks = [[] for _ in range(NBLK)]
                for cl in qproj_closures(0, qsinks[0]):
                    cl()
                fillers.extend(qproj_closures(1, qsinks[1]))

                pend = None
                blk_ots = [[] for _ in range(NBLK)]
                for blk in range(NBLK):
                    if blk + 2 < NBLK:
                        fillers.extend(qproj_closures(blk + 2,
                                                      qsinks[blk + 2]))
                    # the q tiles for this block must be emitted by now
                    while len(qsinks[blk]) < HPC:
                        fill(1)
                    qts = qsinks[blk]
                    for h in range(HPC):
                        exs = []
                        for g in range(NG):
                            ps = ps_sc.tile([P, scw], F32, name="ps_sct")
                            for j in range(SPG):
                                sk = g * SPG + j
                                nc.tensor.matmul(
                                    ps[:, j * SB:(j + 1) * SB],
                                    kT[h][:, sk * P:(sk + 1) * P],
                                    qts[h][:, :],
                                    start=True, stop=True)
                                if pend is not None:
                                    step_pv(pend, sk)
                                fill(2)
                            ext = expp.tile([P, scw], BF16, name="ext")
                            if mask_trivial:
                                nc.scalar.activation(
                                    ext[:, :], ps[:, :],
                                    mybir.ActivationFunctionType.Exp,
                                    bias=0.0, scale=1.0)
                            else:
                                for j in range(SPG):
                                    sk = g * SPG + j
                                    nc.scalar.activation(
                                        ext[:, j * SB:(j + 1) * SB],
                                        ps[:, j * SB:(j + 1) * SB],
                                        mybir.ActivationFunctionType.Exp,
                                        bias=mask_sb[:, sk:sk + 1], scale=1.0)
                            exs.append(ext)
                        if pend is not None:
                            finish_pv_r(pend)
                        pend = start_pv_r(blk, h, exs, qts[h], blk_ots[blk])
                        emit_rsum(pend)
                # drain the last head (blk3/h3): no more scores to interleave
                for sk in range(ST):
                    step_pv(pend, sk)
                    fill(2)
                fill(len(fillers))
                finish_pv_r(pend)
                # the final finish just queued yout(blk3): emit it plain
                fill(len(fillers))

    nc.compile()
    return nc


_NC_CACHE = {}


def _get_nc(has_bias: bool, mask_trivial: bool):
    key = (has_bias, mask_trivial)
    if key not in _NC_CACHE:
        _NC_CACHE[key] = _build(has_bias, mask_trivial)
    return _NC_CACHE[key]


def kernel(hidden_states, attention_mask, Wq, bq, Wk, bk, Wv, bv, Wo, bo):
    hidden_states = np.asarray(hidden_states, dtype=np.float32)
    attention_mask = np.asarray(attention_mask, dtype=np.float32)
    Wq = np.asarray(Wq, dtype=np.float32)
    Wk = np.asarray(Wk, dtype=np.float32)
    Wv = np.asarray(Wv, dtype=np.float32)
    Wo = np.asarray(Wo, dtype=np.float32)
    bq = np.asarray(bq, dtype=np.float32)
    bk = np.asarray(bk, dtype=np.float32)
    bv = np.asarray(bv, dtype=np.float32)
    bo = np.asarray(bo, dtype=np.float32)

    has_bias = bool(np.any(bq) or np.any(bk) or np.any(bv))
    mask_trivial = bool(np.all(attention_mask == 1.0))
    nc = _get_nc(has_bias, mask_trivial)

    # Host-side sharding prep (cheap numpy work, not on the HW critical path)
    xT = [np.ascontiguousarray(hidden_states[b].T).astype(NPBF16)
          for b in range(B)]
    addmask = [np.ascontiguousarray((1.0 - attention_mask[b]) * MASK_MIN)
               for b in range(B)]
    ones2 = np.ones((P, SB), dtype=NPBF16)
    Wq_s = Wq * SCALE          # fold the 1/sqrt(hd) scale into Wq (and bq)
    in_maps = []
    for c in range(N_CORES):
        b, g = c // 4, c % 4
        sl = slice(g * E, (g + 1) * E)
        im = {
            "xT": xT[b],
            "wqT": np.ascontiguousarray(Wq_s[sl, :].T).astype(NPBF16),
            "wkT": np.ascontiguousarray(Wk[sl, :].T).astype(NPBF16),
            "wvT": np.ascontiguousarray(Wv[sl, :].T).astype(NPBF16),
            "woT": np.ascontiguousarray(Wo[:, sl].T).astype(NPBF16),
            "maskT": addmask[b],
            "ones2": ones2,
        }
        if has_bias:
            im["bq"] = np.ascontiguousarray(bq[sl] * SCALE).astype(NPBF16)
            im["bk"] = np.ascontiguousarray(bk[sl]).astype(NPBF16)
            im["bv"] = np.ascontiguousarray(bv[sl]).astype(NPBF16)
        in_maps.append(im)

    res = bass_utils.run_bass_kernel_spmd(
        nc, in_maps, core_ids=list(range(N_CORES)),
        trace=bool(int(os.environ.get("BASS_KERNEL_TRACE", "0"))))
    kernel.last_results = res

    out = np.empty((B, S, D), dtype=np.float32)
    for b in range(B):
        acc = res.results[b * 4]["yT"].astype(np.float32)
        for g in range(1, 4):
            acc += res.results[b * 4 + g]["yT"].astype(np.float32)
        out[b] = acc.T + bo
    return out


# revision 28
# speedup vs baseline: 1.8506x; 1.0399x over previous
"""LlamaAttention (B=2, S=2048, D=2048, H=16) on 8 Trainium2 NeuronCores.

Sharding: batch x head-group. Core c handles batch b = c // 4 and head group
g = c % 4 (4 heads of 128 dims each -> a 512-wide slice of q/k/v space).
Each core computes q/k/v projections for its slice, attention for its 4
heads, and a partial out-projection (contracting only its 512 dv dims).
Host sums the 4 partials per batch and adds the output bias.

v4 design (v1 fp32r ~812us, v2 bf16 ~533us, v3 ~485us):
  - all matmul inputs bf16 (PE issues a [128,128]x[128,512] matmul every
    216ns back-to-back; fp32r took 466ns). PSUM accumulation stays fp32.
  - x and wo resident in SBUF. Weight loads use wide descriptors
    ([128,512] / [128,2048] tiles, stationary operands sliced out of
    them) -- the ~600ns fixed cost per DMA_DIRECT2D dominated the v3
    startup; wk split across the sync+scalar queues, x on gpsimd.
  - the PE queue is kept dense through the scalar-paced exp stretches by
    a single global filler deque: v-projection chunks 2-3, q-projection
    chunks 1-3 and every out-projection block are chopped into
    per-matmul closures and popped 2 per score-slot. The PV/r
    accumulation of head h is interleaved into head h+1's score slots
    and carries across block boundaries.
  - exp fused 4 key-tiles wide: scores land in a 4-bank [128,2048] PSUM
    tile, one ACTIVATE per group (amortizes the 352-elem fixed overhead
    and the per-instruction semaphore 4x). Only valid with an all-ones
    attention mask (ACT bias is per-partition); a general-mask variant
    with per-tile bias is built lazily on first use.
  - softmax denominator: 15 bf16 pairwise/tree adds on DVE, one
    ones-stationary matmul folds the final partition reduction AND the
    broadcast (fp32 PSUM), reciprocal_approx_fast (~0.7us vs 3.4us for
    DVE reciprocal; 51-ULP error is far below the bf16 noise floor),
    normalization fused into the PV-psum eviction.
  - output yT in bf16; host sums the 4 partials per batch in fp32.
"""

import os
import numpy as np
import ml_dtypes

import concourse.bass as bass
import concourse.tile as tile
from concourse import bacc, mybir
from concourse import bass_utils

B, S, D = 2, 2048, 2048
NH, HD = 16, 128
N_CORES = 8
HPC = 4                      # heads per core
E = HPC * HD                 # 512: per-core q/k/v width
SCALE = float(HD) ** -0.5
F32 = mybir.dt.float32
BF16 = mybir.dt.bfloat16

P = 128                      # partition tile
ST = S // P                  # 16 s partition-tiles
DTI = 16                     # d partition-tiles
SB = 512                     # matmul moving-dim block / query block
NBLK = S // SB               # 4 s blocks
MASK_MIN = float(np.finfo(np.float32).min)
NPBF16 = ml_dtypes.bfloat16


def _build(has_bias: bool, mask_trivial: bool):
    nc = bacc.Bacc("TRN2", target_bir_lowering=False, debug=False,
                   num_devices=N_CORES)

    xTd = nc.dram_tensor("xT", [D, S], BF16, kind="ExternalInput").ap()
    wqT = nc.dram_tensor("wqT", [D, E], BF16, kind="ExternalInput").ap()
    wkT = nc.dram_tensor("wkT", [D, E], BF16, kind="ExternalInput").ap()
    wvT = nc.dram_tensor("wvT", [D, E], BF16, kind="ExternalInput").ap()
    woT = nc.dram_tensor("woT", [E, D], BF16, kind="ExternalInput").ap()
    maskT = nc.dram_tensor("maskT", [S], F32, kind="ExternalInput").ap()
    ones2 = nc.dram_tensor("ones2", [P, SB], BF16, kind="ExternalInput").ap()
    if has_bias:
        bqd = nc.dram_tensor("bq", [E], BF16, kind="ExternalInput").ap()
        bkd = nc.dram_tensor("bk", [E], BF16, kind="ExternalInput").ap()
        bvd = nc.dram_tensor("bv", [E], BF16, kind="ExternalInput").ap()
    yT = nc.dram_tensor("yT", [D, S], BF16, kind="ExternalOutput").ap()

    scw = 4 * SB if mask_trivial else 2 * SB   # exp-fusion width
    SPG = scw // SB                            # key tiles per score group
    NG = ST // SPG                             # groups per head-block

    with tile.TileContext(nc) as tc:
        with tc.tile_pool(name="persist", bufs=1) as persist, \
             tc.tile_pool(name="xpool", bufs=1) as xpool:
            kT = [persist.tile([P, S], BF16, name=f"kT{i}", tag=f"kT{i}")
                  for i in range(HPC)]
            vv = [persist.tile([P, E], BF16, name=f"v{i}", tag=f"v{i}")
                  for i in range(ST)]
            mask_sb = persist.tile([P, ST], F32, name="mask_sb", tag="mask")
            nc.sync.dma_start(mask_sb[:, :],
                              maskT.rearrange("(t p) -> p t", p=P))
            ones_sb = persist.tile([P, P], BF16, name="ones_sb", tag="ones")
            nc.sync.dma_start(ones_sb[:, :], ones2[:, 0:P])
            if has_bias:
                ones_row = persist.tile([1, SB], BF16, name="ones_row",
                                        tag="onesr")
                nc.sync.dma_start(ones_row[:, :], ones2[0:1, :])
                ones_rp = persist.tile([1, P], BF16, name="ones_rp",
                                       tag="onesrp")
                nc.sync.dma_start(ones_rp[:, :], ones2[0:1, 0:P])
                bq_sb = persist.tile([1, E], BF16, name="bq_sb", tag="bq")
                bk_sb = persist.tile([1, E], BF16, name="bk_sb", tag="bk")
                bv_sb = persist.tile([1, E], BF16, name="bv_sb", tag="bv")
                nc.sync.dma_start(bq_sb[:, :], bqd.rearrange("(a e) -> a e", a=1))
                nc.sync.dma_start(bk_sb[:, :], bkd.rearrange("(a e) -> a e", a=1))
                nc.sync.dma_start(bv_sb[:, :], bvd.rearrange("(a e) -> a e", a=1))
            # wo resident: 4 wide tiles [128, 2048], sliced per eo later.
            wo_sb = [persist.tile([P, D], BF16, name=f"wo_{dv}",
                                  tag=f"wo_{dv}") for dv in range(HPC)]

            # x resident, chunk 0 first (k-projection needs it immediately);
            # the rest split across the gpsimd and sync queues.
            xx = [xpool.tile([P, S], BF16, name=f"x{dt}", tag=f"x{dt}")
                  for dt in range(DTI)]
            for dt in range(DTI):
                nc.gpsimd.dma_start(xx[dt][:, 0:SB],
                                    xTd[dt * P:(dt + 1) * P, 0:SB])

            # ---------------- Phase K: k projection ----------------
            # weight loads use wide [128,512] descriptors; stationary
            # operands are sliced out of them. wk split sync/scalar queues.
            kvps = tc.tile_pool(name="ps_kv", bufs=4, space="PSUM")
            pskv = kvps.__enter__()
            wkp = tc.tile_pool(name="wkp", bufs=1)
            wkpool = wkp.__enter__()
            wk_sb = []
            for dt in range(DTI):
                w_t = wkpool.tile([P, E], BF16, name=f"wk_{dt}",
                                  tag=f"wk_{dt}")
                q = nc.scalar if dt % 2 == 0 else nc.sync
                q.dma_start(w_t[:, :], wkT[dt * P:(dt + 1) * P, :])
                wk_sb.append(w_t)
            # x chunks 1-3, behind wk on the queues they share
            xq = [nc.gpsimd, nc.sync]
            for dt in range(DTI):
                xq[dt % 2].dma_start(xx[dt][:, SB:S],
                                     xTd[dt * P:(dt + 1) * P, SB:S])
            with nc.named_scope("proj_k"):
                for ch in range(NBLK):
                    c0 = ch * SB
                    pss = [pskv.tile([P, SB], F32, name="ps_k")
                           for _ in range(HPC)]
                    for dt in range(DTI):
                        for et in range(HPC):
                            nc.tensor.matmul(
                                pss[et][:, :],
                                wk_sb[dt][:, et * P:(et + 1) * P],
                                xx[dt][:, c0:c0 + SB],
                                start=(dt == 0),
                                stop=(dt == DTI - 1 and not has_bias))
                    for et in range(HPC):
                        if has_bias:
                            nc.tensor.matmul(
                                pss[et][:, :],
                                bk_sb[0:1, et * P:(et + 1) * P],
                                ones_row[0:1, 0:SB],
                                start=False, stop=True)
                        nc.scalar.copy(kT[et][:, c0:c0 + SB], pss[et][:, :])
            wkp.__exit__(None, None, None)

            # ---------------- Phase V: v projection ----------------
            wvp = tc.tile_pool(name="wvp", bufs=1)
            wvpool = wvp.__enter__()
            wv_sb = []
            for dt in range(DTI):
                w_t = wvpool.tile([P, E], BF16, name=f"wv_{dt}",
                                  tag=f"wv_{dt}")
                nc.sync.dma_start(w_t[:, :], wvT[dt * P:(dt + 1) * P, :])
                wv_sb.append(w_t)
            with nc.named_scope("proj_v"):
                for ch in range(NBLK):
                    c0 = ch * SB
                    pss = [pskv.tile([P, E], F32, name="ps_k")
                           for _ in range(HPC)]
                    for dt in range(DTI):
                        for sl in range(HPC):
                            nc.tensor.matmul(
                                pss[sl][:, :],
                                xx[dt][:, c0 + sl * P:c0 + (sl + 1) * P],
                                wv_sb[dt][:, :],
                                start=(dt == 0),
                                stop=(dt == DTI - 1 and not has_bias))
                    for sl in range(HPC):
                        st = ch * HPC + sl
                        if has_bias:
                            nc.tensor.matmul(
                                pss[sl][:, :], ones_rp[0:1, :],
                                bv_sb[0:1, :], start=False, stop=True)
                        nc.scalar.copy(vv[st][:, :], pss[sl][:, :])
            wvp.__exit__(None, None, None)
            kvps.__exit__(None, None, None)

            # ---------------- Phase QA: q proj + attention + yout ----------
            with nc.named_scope("attn"), \
                 tc.tile_pool(name="wqp", bufs=1) as wqpool, \
                 tc.tile_pool(name="qtn", bufs=2 * HPC) as qpool, \
                 tc.tile_pool(name="otn", bufs=2 * HPC) as opool, \
                 tc.tile_pool(name="expp", bufs=9 * (4 * SB // scw)) as expp, \
                 tc.tile_pool(name="prtp", bufs=10) as prtp, \
                 tc.tile_pool(name="smx", bufs=1) as smx, \
                 tc.tile_pool(name="stage", bufs=2) as stagep, \
                 tc.tile_pool(name="ps_q", bufs=1, space="PSUM") as ps_q, \
                 tc.tile_pool(name="ps_sc", bufs=4 * SB // scw,
                              space="PSUM") as ps_sc, \
                 tc.tile_pool(name="ps_att", bufs=1, space="PSUM") as ps_att:
                wq_sb = []
                for dt in range(DTI):
                    w_t = wqpool.tile([P, E], BF16, name=f"wq_{dt}",
                                      tag=f"wq_{dt}")
                    nc.sync.dma_start(w_t[:, :], wqT[dt * P:(dt + 1) * P, :])
                    wq_sb.append(w_t)
                for dv in range(HPC):
                    nc.sync.dma_start(wo_sb[dv][:, :],
                                      woT[dv * P:(dv + 1) * P, :])

                def qproj_closures(blk, sink):
                    c0 = blk * SB
                    cls = []
                    for et in range(HPC):
                        hold = {}
                        for dt in range(DTI):
                            def cl(et=et, dt=dt, c0=c0, hold=hold, sink=sink):
                                if dt == 0:
                                    hold["ps"] = ps_q.tile([P, SB], F32,
                                                           name="ps_qt")
                                ps = hold["ps"]
                                nc.tensor.matmul(
                                    ps[:, :],
                                    wq_sb[dt][:, et * P:(et + 1) * P],
                                    xx[dt][:, c0:c0 + SB],
                                    start=(dt == 0),
                                    stop=(dt == DTI - 1 and not has_bias))
                                if dt == DTI - 1:
                                    if has_bias:
                                        nc.tensor.matmul(
                                            ps[:, :],
                                            bq_sb[0:1, et * P:(et + 1) * P],
                                            ones_row[0:1, 0:SB],
                                            start=False, stop=True)
                                    qt = qpool.tile([P, SB], BF16, name="qt")
                                    nc.scalar.copy(qt[:, :], ps[:, :])
                                    sink.append(qt)
                            cls.append(cl)
                    return cls

                def yout_closures(blk, ots):
                    q0 = blk * SB
                    cls = []
                    for eo in range(DTI):
                        hold = {}
                        for dv in range(HPC):
                            def cl(eo=eo, dv=dv, q0=q0, hold=hold, ots=ots):
                                if dv == 0:
                                    hold["ps"] = ps_q.tile([P, SB], F32,
                                                           name="yps")
                                yps = hold["ps"]
                                nc.tensor.matmul(
                                    yps[:, :],
                                    wo_sb[dv][:, eo * P:(eo + 1) * P],
                                    ots[dv][:, :],
                                    start=(dv == 0), stop=(dv == HPC - 1))
                                if dv == HPC - 1:
                                    stg = stagep.tile([P, SB], BF16,
                                                      name="stg")
                                    # alternate evict engine and store queue
                                    # so the final block's drain isn't one
                                    # serial chain
                                    if eo % 2 == 0:
                                        nc.vector.tensor_copy(stg[:, :],
                                                              yps[:, :])
                                        nc.gpsimd.dma_start(
                                            yT[eo * P:(eo + 1) * P,
                                               q0:q0 + SB], stg[:, :])
                                    else:
                                        nc.scalar.copy(stg[:, :], yps[:, :])
                                        nc.sync.dma_start(
                                            yT[eo * P:(eo + 1) * P,
                                               q0:q0 + SB], stg[:, :])
                            cls.append(cl)
                    return cls

                fillers = []
                fidx = [0]

                def fill(n):
                    while n > 0 and fidx[0] < len(fillers):
                        fillers[fidx[0]]()
                        fidx[0] += 1
                        n -= 1

                def start_pv_r(blk, h, exs, qt, ots):
                    ops = ps_att.tile([P, SB], F32, name="ops")
                    rps = ps_att.tile([P, SB], F32, name="rps")
                    return {"blk": blk, "h": h, "exs": exs, "qt": qt,
                            "ops": ops, "rps": rps, "racc": None, "ots": ots}

                def step_pv(pend, sk):
                    h = pend["h"]
                    big, off = pend["exs"][sk * SB // scw], (sk * SB) % scw
                    nc.tensor.matmul(
                        pend["ops"][:, :],
                        vv[sk][:, h * P:(h + 1) * P],
                        big[:, off:off + SB],
                        start=(sk == 0), stop=(sk == ST - 1))

                def emit_rsum(pend):
                    lvl = []
                    for exb in pend["exs"]:
                        for o in range(0, scw, 2 * SB):
                            pt = prtp.tile([P, SB], BF16, name="prt")
                            nc.vector.tensor_add(pt[:, :],
                                                 exb[:, o:o + SB],
                                                 exb[:, o + SB:o + 2 * SB])
                            lvl.append(pt)
                    while len(lvl) > 1:
                        nxt = []
                        for i in range(0, len(lvl), 2):
                            pt = prtp.tile([P, SB], BF16, name="prt")
                            nc.vector.tensor_add(pt[:, :], lvl[i][:, :],
                                                 lvl[i + 1][:, :])
                            nxt.append(pt)
                        lvl = nxt
                    pend["racc"] = lvl[0]

                def finish_pv_r(pend):
                    # ones-matmul emitted at finish time so the PE queue never
                    # waits on the DVE add tree (it ran during the next head's
                    # scores).
                    nc.tensor.matmul(pend["rps"][:, :], ones_sb[:, :],
                                     pend["racc"][:, :], start=True, stop=True)
                    rbc = smx.tile([P, SB], F32, name="rbc")
                    nc.vector.reciprocal_approx_fast(rbc[:, :],
                                                     pend["rps"][:, :])
                    ot = opool.tile([P, SB], BF16, name="ot")
                    nc.vector.tensor_mul(ot[:, :], pend["ops"][:, :],
                                         rbc[:, :])
                    pend["ots"].append(ot)
                    if pend["h"] == HPC - 1:
                        fillers.extend(yout_closures(pend["blk"], pend["ots"]))

                # q chunk 0 emitted plain; q1 seeds the deque
                qsinks = [[] for _ in range(NBLK)]
                for cl in qproj_closures(0, qsinks[0]):
                    cl()
                fillers.extend(qproj_closures(1, qsinks[1]))

                pend = None
                blk_ots = [[] for _ in range(NBLK)]
                for blk in range(NBLK):
                    if blk + 2 < NBLK:
                        fillers.extend(qproj_closures(blk + 2,
                                                      qsinks[blk + 2]))
                    # the q tiles for this block must be emitted by now
                    while len(qsinks[blk]) < HPC:
                        fill(1)
                    qts = qsinks[blk]
                    for h in range(HPC):
                        exs = []
                        for g in range(NG):
                            ps = ps_sc.tile([P, scw], F32, name="ps_sct")
                            for j in range(SPG):
                                slot = g * SPG + j
                                nc.tensor.matmul(
                                    ps[:, j * SB:(j + 1) * SB],
                                    kT[h][:, slot * P:(slot + 1) * P],
                                    qts[h][:, :],
                                    start=True, stop=True)
                                # prev head's PV chain runs 2-per-slot in the
                                # first half so its psum bank + the normalize
                                # chain drain long before the next head needs
                                # them; fillers take the second half.
                                if pend is not None and slot < ST // 2:
                                    step_pv(pend, 2 * slot)
                                    step_pv(pend, 2 * slot + 1)
                                else:
                                    fill(2)
                                if slot >= ST // 2:
                                    fill(1)
                                if slot == 12 and pend is not None:
                                    finish_pv_r(pend)
                                    pend = None
                            ext = expp.tile([P, scw], BF16, name="ext")
                            if mask_trivial:
                                nc.scalar.activation(
                                    ext[:, :], ps[:, :],
                                    mybir.ActivationFunctionType.Exp,
                                    bias=0.0, scale=1.0)
                            else:
                                for j in range(SPG):
                                    sk = g * SPG + j
                                    nc.scalar.activation(
                                        ext[:, j * SB:(j + 1) * SB],
                                        ps[:, j * SB:(j + 1) * SB],
                                        mybir.ActivationFunctionType.Exp,
                                        bias=mask_sb[:, sk:sk + 1], scale=1.0)
                            exs.append(ext)
                        pend = start_pv_r(blk, h, exs, qts[h], blk_ots[blk])
                        emit_rsum(pend)
                # drain the last head (blk3/h3): no more scores to interleave
                for sk in range(ST):
                    step_pv(pend, sk)
                    fill(2)
                fill(len(fillers))
                finish_pv_r(pend)
                # the final finish just queued yout(blk3): emit it plain
                fill(len(fillers))

    nc.compile()
    return nc


_NC_CACHE = {}


def _get_nc(has_bias: bool, mask_trivial: bool):
    key = (has_bias, mask_trivial)
    if key not in _NC_CACHE:
        _NC_CACHE[key] = _build(has_bias, mask_trivial)
    return _NC_CACHE[key]


def kernel(hidden_states, attention_mask, Wq, bq, Wk, bk, Wv, bv, Wo, bo):
    hidden_states = np.asarray(hidden_states, dtype=np.float32)
    attention_mask = np.asarray(attention_mask, dtype=np.float32)
    Wq = np.asarray(Wq, dtype=np.float32)
    Wk = np.asarray(Wk, dtype=np.float32)
    Wv = np.asarray(Wv, dtype=np.float32)
    Wo = np.asarray(Wo, dtype=np.float32)
    bq = np.asarray(bq, dtype=np.float32)
    bk = np.asarray(bk, dtype=np.float32)
    bv = np.asarray(bv, dtype=np.float32)
    bo = np.asarray(bo, dtype=np.float32)

    has_bias = bool(np.any(bq) or np.any(bk) or np.any(bv))
    mask_trivial = bool(np.all(attention_mask == 1.0))
    nc = _get_nc(has_bias, mask_trivial)

    # Host-side sharding prep (cheap numpy work, not on the HW critical path)
    xT = [np.ascontiguousarray(hidden_states[b].T).astype(NPBF16)
          for b in range(B)]
    addmask = [np.ascontiguousarray((1.0 - attention_mask[b]) * MASK_MIN)
               for b in range(B)]
    ones2 = np.ones((P, SB), dtype=NPBF16)
    Wq_s = Wq * SCALE          # fold the 1/sqrt(hd) scale into Wq (and bq)
    in_maps = []
    for c in range(N_CORES):
        b, g = c // 4, c % 4
        sl = slice(g * E, (g + 1) * E)
        im = {
            "xT": xT[b],
            "wqT": np.ascontiguousarray(Wq_s[sl, :].T).astype(NPBF16),
            "wkT": np.ascontiguousarray(Wk[sl, :].T).astype(NPBF16),
            "wvT": np.ascontiguousarray(Wv[sl, :].T).astype(NPBF16),
            "woT": np.ascontiguousarray(Wo[:, sl].T).astype(NPBF16),
            "maskT": addmask[b],
            "ones2": ones2,
        }
        if has_bias:
            im["bq"] = np.ascontiguousarray(bq[sl] * SCALE).astype(NPBF16)
            im["bk"] = np.ascontiguousarray(bk[sl]).astype(NPBF16)
            im["bv"] = np.ascontiguousarray(bv[sl]).astype(NPBF16)
        in_maps.append(im)

    res = bass_utils.run_bass_kernel_spmd(
        nc, in_maps, core_ids=list(range(N_CORES)),
        trace=bool(int(os.environ.get("BASS_KERNEL_TRACE", "0"))))
    kernel.last_results = res

    out = np.empty((B, S, D), dtype=np.float32)
    for b in range(B):
        acc = res.results[b * 4]["yT"].astype(np.float32)
        for g in range(1, 4):
            acc += res.results[b * 4 + g]["yT"].astype(np.float32)
        out[b] = acc.T + bo
    return out


# revision 34
# speedup vs baseline: 1.9463x; 1.0517x over previous
"""LlamaAttention (B=2, S=2048, D=2048, H=16) on 8 Trainium2 NeuronCores.

Sharding: batch x head-group. Core c handles batch b = c // 4 and head group
g = c % 4 (4 heads of 128 dims each -> a 512-wide slice of q/k/v space).
Each core computes q/k/v projections for its slice, attention for its 4
heads, and a partial out-projection (contracting only its 512 dv dims).
Host sums the 4 partials per batch and adds the output bias.

v4 design (v1 fp32r ~812us, v2 bf16 ~533us, v3 ~485us):
  - all matmul inputs bf16 (PE issues a [128,128]x[128,512] matmul every
    216ns back-to-back; fp32r took 466ns). PSUM accumulation stays fp32.
  - x and wo resident in SBUF. Weight loads use wide descriptors
    ([128,512] / [128,2048] tiles, stationary operands sliced out of
    them) -- the ~600ns fixed cost per DMA_DIRECT2D dominated the v3
    startup; wk split across the sync+scalar queues, x on gpsimd.
  - the PE queue is kept dense through the scalar-paced exp stretches by
    a single global filler deque: v-projection chunks 2-3, q-projection
    chunks 1-3 and every out-projection block are chopped into
    per-matmul closures and popped 2 per score-slot. The PV/r
    accumulation of head h is interleaved into head h+1's score slots
    and carries across block boundaries.
  - exp fused 4 key-tiles wide: scores land in a 4-bank [128,2048] PSUM
    tile, one ACTIVATE per group (amortizes the 352-elem fixed overhead
    and the per-instruction semaphore 4x). Only valid with an all-ones
    attention mask (ACT bias is per-partition); a general-mask variant
    with per-tile bias is built lazily on first use.
  - softmax denominator: 15 bf16 pairwise/tree adds on DVE, one
    ones-stationary matmul folds the final partition reduction AND the
    broadcast (fp32 PSUM), reciprocal_approx_fast (~0.7us vs 3.4us for
    DVE reciprocal; 51-ULP error is far below the bf16 noise floor),
    normalization fused into the PV-psum eviction.
  - output yT in bf16; host sums the 4 partials per batch in fp32.
"""

import os
import numpy as np
import ml_dtypes

import concourse.bass as bass
import concourse.tile as tile
from concourse import bacc, mybir
from concourse import bass_utils

B, S, D = 2, 2048, 2048
NH, HD = 16, 128
N_CORES = 8
HPC = 4                      # heads per core
E = HPC * HD                 # 512: per-core q/k/v width
SCALE = float(HD) ** -0.5
F32 = mybir.dt.float32
BF16 = mybir.dt.bfloat16

P = 128                      # partition tile
ST = S // P                  # 16 s partition-tiles
DTI = 16                     # d partition-tiles
SB = 512                     # matmul moving-dim block / query block
NBLK = S // SB               # 4 s blocks
MASK_MIN = float(np.finfo(np.float32).min)
NPBF16 = ml_dtypes.bfloat16


def _build(has_bias: bool, mask_trivial: bool):
    nc = bacc.Bacc("TRN2", target_bir_lowering=False, debug=False,
                   num_devices=N_CORES)

    xTd = nc.dram_tensor("xT", [D, S], BF16, kind="ExternalInput").ap()
    wqT = nc.dram_tensor("wqT", [D, E], BF16, kind="ExternalInput").ap()
    wkT = nc.dram_tensor("wkT", [D, E], BF16, kind="ExternalInput").ap()
    wvT = nc.dram_tensor("wvT", [D, E], BF16, kind="ExternalInput").ap()
    woT = nc.dram_tensor("woT", [E, D], BF16, kind="ExternalInput").ap()
    maskT = nc.dram_tensor("maskT", [S], F32, kind="ExternalInput").ap()
    ones2 = nc.dram_tensor("ones2", [P, SB], BF16, kind="ExternalInput").ap()
    if has_bias:
        bqd = nc.dram_tensor("bq", [E], BF16, kind="ExternalInput").ap()
        bkd = nc.dram_tensor("bk", [E], BF16, kind="ExternalInput").ap()
        bvd = nc.dram_tensor("bv", [E], BF16, kind="ExternalInput").ap()
    yT = nc.dram_tensor("yT", [D, S], BF16, kind="ExternalOutput").ap()

    scw = 4 * SB if mask_trivial else 2 * SB   # exp-fusion width
    SPG = scw // SB                            # key tiles per score group
    NG = ST // SPG                             # groups per head-block

    with tile.TileContext(nc) as tc:
        with tc.tile_pool(name="persist", bufs=1) as persist, \
             tc.tile_pool(name="xpool", bufs=1) as xpool:
            kT = [persist.tile([P, S], BF16, name=f"kT{i}", tag=f"kT{i}")
                  for i in range(HPC)]
            vv = [persist.tile([P, E], BF16, name=f"v{i}", tag=f"v{i}")
                  for i in range(ST)]
            mask_sb = persist.tile([P, ST], F32, name="mask_sb", tag="mask")
            nc.sync.dma_start(mask_sb[:, :],
                              maskT.rearrange("(t p) -> p t", p=P))
            ones_sb = persist.tile([P, P], BF16, name="ones_sb", tag="ones")
            nc.sync.dma_start(ones_sb[:, :], ones2[:, 0:P])
            if has_bias:
                ones_row = persist.tile([1, SB], BF16, name="ones_row",
                                        tag="onesr")
                nc.sync.dma_start(ones_row[:, :], ones2[0:1, :])
                ones_rp = persist.tile([1, P], BF16, name="ones_rp",
                                       tag="onesrp")
                nc.sync.dma_start(ones_rp[:, :], ones2[0:1, 0:P])
                bq_sb = persist.tile([1, E], BF16, name="bq_sb", tag="bq")
                bk_sb = persist.tile([1, E], BF16, name="bk_sb", tag="bk")
                bv_sb = persist.tile([1, E], BF16, name="bv_sb", tag="bv")
                nc.sync.dma_start(bq_sb[:, :], bqd.rearrange("(a e) -> a e", a=1))
                nc.sync.dma_start(bk_sb[:, :], bkd.rearrange("(a e) -> a e", a=1))
                nc.sync.dma_start(bv_sb[:, :], bvd.rearrange("(a e) -> a e", a=1))
            # wo resident: 4 wide tiles [128, 2048], sliced per eo later.
            wo_sb = [persist.tile([P, D], BF16, name=f"wo_{dv}",
                                  tag=f"wo_{dv}") for dv in range(HPC)]

            # x resident, chunk 0 first (k-projection needs it immediately);
            # the rest split across the gpsimd and sync queues.
            xx = [xpool.tile([P, S], BF16, name=f"x{dt}", tag=f"x{dt}")
                  for dt in range(DTI)]
            for dt in range(DTI):
                nc.gpsimd.dma_start(xx[dt][:, 0:SB],
                                    xTd[dt * P:(dt + 1) * P, 0:SB])

            # ---------------- Phase K: k projection ----------------
            # weight loads use wide [128,512] descriptors; stationary
            # operands are sliced out of them. wk split sync/scalar queues.
            # All weight pools are opened up front in fresh SBUF (wqp first,
            # LIFO) so no weight DMA has a write-after-read hazard against a
            # projection still reading the previous pool's space.
            wqp = tc.tile_pool(name="wqp", bufs=1)
            wqpool = wqp.__enter__()
            wkp = tc.tile_pool(name="wkp", bufs=1)
            wkpool = wkp.__enter__()
            wvp = tc.tile_pool(name="wvp", bufs=1)
            wvpool = wvp.__enter__()
            kvps = tc.tile_pool(name="ps_kv", bufs=8, space="PSUM")
            pskv = kvps.__enter__()
            wk_sb, wv_sb, wq_sb = [], [], []
            for dt in range(DTI):
                w_t = wkpool.tile([P, E], BF16, name=f"wk_{dt}",
                                  tag=f"wk_{dt}")
                q = nc.scalar if dt % 2 == 0 else nc.sync
                q.dma_start(w_t[:, :], wkT[dt * P:(dt + 1) * P, :])
                wk_sb.append(w_t)
            # x chunks 1-3, behind wk on the queues they share
            xq = [nc.gpsimd, nc.sync]
            for dt in range(DTI):
                xq[dt % 2].dma_start(xx[dt][:, SB:S],
                                     xTd[dt * P:(dt + 1) * P, SB:S])
            for dt in range(DTI):
                w_t = wvpool.tile([P, E], BF16, name=f"wv_{dt}",
                                  tag=f"wv_{dt}")
                nc.sync.dma_start(w_t[:, :], wvT[dt * P:(dt + 1) * P, :])
                wv_sb.append(w_t)
            for dt in range(DTI):
                w_t = wqpool.tile([P, E], BF16, name=f"wq_{dt}",
                                  tag=f"wq_{dt}")
                nc.sync.dma_start(w_t[:, :], wqT[dt * P:(dt + 1) * P, :])
                wq_sb.append(w_t)
            for dv in range(HPC):
                nc.sync.dma_start(wo_sb[dv][:, :],
                                  woT[dv * P:(dv + 1) * P, :])
            with nc.named_scope("proj_k"):
                for ch in range(NBLK):
                    c0 = ch * SB
                    pss = [pskv.tile([P, SB], F32, name="ps_k")
                           for _ in range(HPC)]
                    for dt in range(DTI):
                        for et in range(HPC):
                            nc.tensor.matmul(
                                pss[et][:, :],
                                wk_sb[dt][:, et * P:(et + 1) * P],
                                xx[dt][:, c0:c0 + SB],
                                start=(dt == 0),
                                stop=(dt == DTI - 1 and not has_bias))
                    for et in range(HPC):
                        if has_bias:
                            nc.tensor.matmul(
                                pss[et][:, :],
                                bk_sb[0:1, et * P:(et + 1) * P],
                                ones_row[0:1, 0:SB],
                                start=False, stop=True)
                        nc.scalar.copy(kT[et][:, c0:c0 + SB], pss[et][:, :])

            # ---------------- Phase V: v projection ----------------
            with nc.named_scope("proj_v"):
                for ch in range(NBLK):
                    c0 = ch * SB
                    pss = [pskv.tile([P, E], F32, name="ps_k")
                           for _ in range(HPC)]
                    for dt in range(DTI):
                        for sl in range(HPC):
                            nc.tensor.matmul(
                                pss[sl][:, :],
                                xx[dt][:, c0 + sl * P:c0 + (sl + 1) * P],
                                wv_sb[dt][:, :],
                                start=(dt == 0),
                                stop=(dt == DTI - 1 and not has_bias))
                    for sl in range(HPC):
                        st = ch * HPC + sl
                        if has_bias:
                            nc.tensor.matmul(
                                pss[sl][:, :], ones_rp[0:1, :],
                                bv_sb[0:1, :], start=False, stop=True)
                        nc.scalar.copy(vv[st][:, :], pss[sl][:, :])
            kvps.__exit__(None, None, None)
            wvp.__exit__(None, None, None)
            wkp.__exit__(None, None, None)

            # ---------------- Phase QA: q proj + attention + yout ----------
            with nc.named_scope("attn"), \
                 tc.tile_pool(name="qtn", bufs=2 * HPC) as qpool, \
                 tc.tile_pool(name="otn", bufs=2 * HPC) as opool, \
                 tc.tile_pool(name="expp", bufs=9 * (4 * SB // scw)) as expp, \
                 tc.tile_pool(name="prtp", bufs=10) as prtp, \
                 tc.tile_pool(name="smx", bufs=1) as smx, \
                 tc.tile_pool(name="stage", bufs=2) as stagep, \
                 tc.tile_pool(name="ps_q", bufs=1, space="PSUM") as ps_q, \
                 tc.tile_pool(name="ps_sc", bufs=4 * SB // scw,
                              space="PSUM") as ps_sc, \
                 tc.tile_pool(name="ps_att", bufs=1, space="PSUM") as ps_att:

                def qproj_closures(blk, sink):
                    c0 = blk * SB
                    cls = []
                    for et in range(HPC):
                        hold = {}
                        for dt in range(DTI):
                            def cl(et=et, dt=dt, c0=c0, hold=hold, sink=sink):
                                if dt == 0:
                                    hold["ps"] = ps_q.tile([P, SB], F32,
                                                           name="ps_qt")
                                ps = hold["ps"]
                                nc.tensor.matmul(
                                    ps[:, :],
                                    wq_sb[dt][:, et * P:(et + 1) * P],
                                    xx[dt][:, c0:c0 + SB],
                                    start=(dt == 0),
                                    stop=(dt == DTI - 1 and not has_bias))
                                if dt == DTI - 1:
                                    if has_bias:
                                        nc.tensor.matmul(
                                            ps[:, :],
                                            bq_sb[0:1, et * P:(et + 1) * P],
                                            ones_row[0:1, 0:SB],
                                            start=False, stop=True)
                                    qt = qpool.tile([P, SB], BF16, name="qt")
                                    nc.scalar.copy(qt[:, :], ps[:, :])
                                    sink.append(qt)
                            cls.append(cl)
                    return cls

                def yout_closures(blk, ots):
                    q0 = blk * SB
                    cls = []
                    for eo in range(DTI):
                        hold = {}
                        for dv in range(HPC):
                            def cl(eo=eo, dv=dv, q0=q0, hold=hold, ots=ots):
                                if dv == 0:
                                    hold["ps"] = ps_q.tile([P, SB], F32,
                                                           name="yps")
                                yps = hold["ps"]
                                nc.tensor.matmul(
                                    yps[:, :],
                                    wo_sb[dv][:, eo * P:(eo + 1) * P],
                                    ots[dv][:, :],
                                    start=(dv == 0), stop=(dv == HPC - 1))
                                if dv == HPC - 1:
                                    stg = stagep.tile([P, SB], BF16,
                                                      name="stg")
                                    # alternate evict engine and store queue
                                    # so the final block's drain isn't one
                                    # serial chain
                                    if eo % 2 == 0:
                                        nc.vector.tensor_copy(stg[:, :],
                                                              yps[:, :])
                                        nc.gpsimd.dma_start(
                                            yT[eo * P:(eo + 1) * P,
                                               q0:q0 + SB], stg[:, :])
                                    else:
                                        nc.scalar.copy(stg[:, :], yps[:, :])
                                        nc.sync.dma_start(
                                            yT[eo * P:(eo + 1) * P,
                                               q0:q0 + SB], stg[:, :])
                            cls.append(cl)
                    return cls

                fillers = []
                fidx = [0]

                def fill(n):
                    while n > 0 and fidx[0] < len(fillers):
                        fillers[fidx[0]]()
                        fidx[0] += 1
                        n -= 1

                def start_pv_r(blk, h, exs, qt, ots):
                    ops = ps_att.tile([P, SB], F32, name="ops")
                    rps = ps_att.tile([P, SB], F32, name="rps")
                    return {"blk": blk, "h": h, "exs": exs, "qt": qt,
                            "ops": ops, "rps": rps, "racc": None, "ots": ots}

                def step_pv(pend, sk):
                    h = pend["h"]
                    big, off = pend["exs"][sk * SB // scw], (sk * SB) % scw
                    nc.tensor.matmul(
                        pend["ops"][:, :],
                        vv[sk][:, h * P:(h + 1) * P],
                        big[:, off:off + SB],
                        start=(sk == 0), stop=(sk == ST - 1))

                def emit_rsum(pend):
                    lvl = []
                    for exb in pend["exs"]:
                        for o in range(0, scw, 2 * SB):
                            pt = prtp.tile([P, SB], BF16, name="prt")
                            nc.vector.tensor_add(pt[:, :],
                                                 exb[:, o:o + SB],
                                                 exb[:, o + SB:o + 2 * SB])
                            lvl.append(pt)
                    while len(lvl) > 1:
                        nxt = []
                        for i in range(0, len(lvl), 2):
                            pt = prtp.tile([P, SB], BF16, name="prt")
                            nc.vector.tensor_add(pt[:, :], lvl[i][:, :],
                                                 lvl[i + 1][:, :])
                            nxt.append(pt)
                        lvl = nxt
                    pend["racc"] = lvl[0]

                def finish_pv_r(pend):
                    # ones-matmul emitted at finish time so the PE queue never
                    # waits on the DVE add tree (it ran during the next head's
                    # scores).
                    nc.tensor.matmul(pend["rps"][:, :], ones_sb[:, :],
                                     pend["racc"][:, :], start=True, stop=True)
                    rbc = smx.tile([P, SB], F32, name="rbc")
                    nc.vector.reciprocal_approx_fast(rbc[:, :],
                                                     pend["rps"][:, :])
                    ot = opool.tile([P, SB], BF16, name="ot")
                    nc.vector.tensor_mul(ot[:, :], pend["ops"][:, :],
                                         rbc[:, :])
                    pend["ots"].append(ot)
                    if pend["h"] == HPC - 1:
                        fillers.extend(yout_closures(pend["blk"], pend["ots"]))

                # q chunk 0 emitted plain; q1 seeds the deque
                qsinks = [[] for _ in range(NBLK)]
                for cl in qproj_closures(0, qsinks[0]):
                    cl()
                fillers.extend(qproj_closures(1, qsinks[1]))

                pend = None
                blk_ots = [[] for _ in range(NBLK)]
                for blk in range(NBLK):
                    if blk + 2 < NBLK:
                        fillers.extend(qproj_closures(blk + 2,
                                                      qsinks[blk + 2]))
                    # the q tiles for this block must be emitted by now
                    while len(qsinks[blk]) < HPC:
                        fill(1)
                    qts = qsinks[blk]
                    for h in range(HPC):
                        exs = []
                        for g in range(NG):
                            ps = ps_sc.tile([P, scw], F32, name="ps_sct")
                            for j in range(SPG):
                                slot = g * SPG + j
                                nc.tensor.matmul(
                                    ps[:, j * SB:(j + 1) * SB],
                                    kT[h][:, slot * P:(slot + 1) * P],
                                    qts[h][:, :],
                                    start=True, stop=True)
                                # prev head's PV chain runs 2-per-slot in the
                                # first half so its psum bank + the normalize
                                # chain drain long before the next head needs
                                # them; fillers take the second half.
                                if pend is not None and slot < ST // 2:
                                    step_pv(pend, 2 * slot)
                                    step_pv(pend, 2 * slot + 1)
                                else:
                                    fill(2)
                                if slot >= ST // 2:
                                    fill(1)
                                if slot == 12 and pend is not None:
                                    finish_pv_r(pend)
                                    pend = None
                            ext = expp.tile([P, scw], BF16, name="ext")
                            if mask_trivial:
                                nc.scalar.activation(
                                    ext[:, :], ps[:, :],
                                    mybir.ActivationFunctionType.Exp,
                                    bias=0.0, scale=1.0)
                            else:
                                for j in range(SPG):
                                    sk = g * SPG + j
                                    nc.scalar.activation(
                                        ext[:, j * SB:(j + 1) * SB],
                                        ps[:, j * SB:(j + 1) * SB],
                                        mybir.ActivationFunctionType.Exp,
                                        bias=mask_sb[:, sk:sk + 1], scale=1.0)
                            exs.append(ext)
                        pend = start_pv_r(blk, h, exs, qts[h], blk_ots[blk])
                        emit_rsum(pend)
                # drain the last head (blk3/h3): no more scores to interleave
                for sk in range(ST):
                    step_pv(pend, sk)
                    fill(2)
                fill(len(fillers))
                finish_pv_r(pend)
                # the final finish just queued yout(blk3): emit it plain
                fill(len(fillers))
            wqp.__exit__(None, None, None)

    nc.compile()
    return nc


_NC_CACHE = {}


def _get_nc(has_bias: bool, mask_trivial: bool):
    key = (has_bias, mask_trivial)
    if key not in _NC_CACHE:
        _NC_CACHE[key] = _build(has_bias, mask_trivial)
    return _NC_CACHE[key]


def kernel(hidden_states, attention_mask, Wq, bq, Wk, bk, Wv, bv, Wo, bo):
    hidden_states = np.asarray(hidden_states, dtype=np.float32)
    attention_mask = np.asarray(attention_mask, dtype=np.float32)
    Wq = np.asarray(Wq, dtype=np.float32)
    Wk = np.asarray(Wk, dtype=np.float32)
    Wv = np.asarray(Wv, dtype=np.float32)
    Wo = np.asarray(Wo, dtype=np.float32)
    bq = np.asarray(bq, dtype=np.float32)
    bk = np.asarray(bk, dtype=np.float32)
    bv = np.asarray(bv, dtype=np.float32)
    bo = np.asarray(bo, dtype=np.float32)

    has_bias = bool(np.any(bq) or np.any(bk) or np.any(bv))
    mask_trivial = bool(np.all(attention_mask == 1.0))
    nc = _get_nc(has_bias, mask_trivial)

    # Host-side sharding prep (cheap numpy work, not on the HW critical path)
    xT = [np.ascontiguousarray(hidden_states[b].T).astype(NPBF16)
          for b in range(B)]
    addmask = [np.ascontiguousarray((1.0 - attention_mask[b]) * MASK_MIN)
               for b in range(B)]
    ones2 = np.ones((P, SB), dtype=NPBF16)
    Wq_s = Wq * SCALE          # fold the 1/sqrt(hd) scale into Wq (and bq)
    in_maps = []
    for c in range(N_CORES):
        b, g = c // 4, c % 4
        sl = slice(g * E, (g + 1) * E)
        im = {
            "xT": xT[b],
            "wqT": np.ascontiguousarray(Wq_s[sl, :].T).astype(NPBF16),
            "wkT": np.ascontiguousarray(Wk[sl, :].T).astype(NPBF16),
            "wvT": np.ascontiguousarray(Wv[sl, :].T).astype(NPBF16),
            "woT": np.ascontiguousarray(Wo[:, sl].T).astype(NPBF16),
            "maskT": addmask[b],
            "ones2": ones2,
        }
        if has_bias:
            im["bq"] = np.ascontiguousarray(bq[sl] * SCALE).astype(NPBF16)
            im["bk"] = np.ascontiguousarray(bk[sl]).astype(NPBF16)
            im["bv"] = np.ascontiguousarray(bv[sl]).astype(NPBF16)
        in_maps.append(im)

    res = bass_utils.run_bass_kernel_spmd(
        nc, in_maps, core_ids=list(range(N_CORES)),
        trace=bool(int(os.environ.get("BASS_KERNEL_TRACE", "0"))))
    kernel.last_results = res

    out = np.empty((B, S, D), dtype=np.float32)
    for b in range(B):
        acc = res.results[b * 4]["yT"].astype(np.float32)
        for g in range(1, 4):
            acc += res.results[b * 4 + g]["yT"].astype(np.float32)
        out[b] = acc.T + bo
    return out
